# revision 43
# baseline (speedup 1.0000x reference)
"""GQA attention layer (B=1, S=2048, D=4096, H=32, KVH=8, HD=128) on 8 TRN2
NeuronCores, tensor-parallel over heads.

Each core computes 4 query heads + their shared kv head end-to-end:
QKV projection -> RoPE -> causal attention (no-max-sub softmax, scores are
tiny) -> its slice of the wo projection. The 8 partial [S, D] outputs are
summed on the host (the "all-reduce after wo" of the sharding hint).

Device layouts (everything bf16 into the PE, fp32 PSUM accumulation):
  QT/KT  [HD=128(part), S]    from  lhsT=w[d,:], rhs=xT[d, s-tile]
  V      [S(part), HD]        via PE-transpose of VT
  scoresT[k(part), q]         lhsT=KT chunk, rhs=QT tile
  E = exp(scoresT/128) bf16; causal diagonal via 0/1 mask multiply
  attnT  [HD(part), q]        lhsT=V chunk, rhs=E  (accumulated over k)
  den    [128, q] bf16 SBUF   accumulated on DVE/GpSimd (off PE)
  den reduce+broadcast        one matmul lhsT=ones[128,128], rhs=den
  attnT_norm = attnT * recip  (DVE mul, bf16 out)
  out    [s(part), n]         lhsT=attnT_norm chunk, rhs=woT

All DMA-heavy tensors are pre-swizzled on the host into exactly the SBUF
tile layout (per-partition contiguous), so every dma_start is a plain
[128, contiguous] block: cheap to dispatch and line-rate to transfer.
x tile loads run several quarters ahead of compute on the sync queue.

The last s-tile runs its QK matmuls head-major so each head's RoPE (and the
PSUM bank it frees for phase B's prefetched q-tile-0 attnV) completes while
later heads' matmuls still stream; q-tile 0's softmax denominators +
reciprocals are also emitted inside the phase-A tail, hiding the ACT
reciprocal table switch.

wo matmuls for q-tile t-1 are interleaved between the scores and attnV
matmuls of q-tile t so the PE fills the exp-wait gaps (the scalar engine's
exps per chunk exceed the attention matmul time per chunk).
"""

import json
import math

import ml_dtypes
import numpy as np

import concourse.bass as bass
import concourse.tile as tile
from concourse import mybir
from concourse.bass_utils import run_bass_kernel_spmd

BF16 = mybir.dt.bfloat16
F32 = mybir.dt.float32
FP8 = mybir.dt.float8e4
NPBF16 = ml_dtypes.bfloat16
NPFP8 = ml_dtypes.float8_e4m3

# Full problem constants
B, S, D = 1, 2048, 4096
H, KVH = 32, 8
HD = 128
NCORES = 8
HQ = H // NCORES  # query heads per core
MULT = 1.0
ROPE_BASE = 10000.0
ST = 512  # s-tile (PSUM bank width in fp32)


def attn_scale(seq_len=S, d_head=HD, mult=MULT):
    alpha = 1.0 / (1.0 + 4.0 * d_head / mult**2)
    lower = (math.log(seq_len) / seq_len) ** 0.5
    interp = math.exp((1.0 - alpha) * math.log(lower))
    return 1.0 / interp


def _legalize_single_wait(nc):
    """The walrus build in this container accepts only ONE sync wait per
    instruction ("Too many sync wait commands" in setupSyncWait). Split
    extra waits into preceding single-wait Drains (lowered to CTRL NOPs)
    on the same engine — same in-order stall semantics."""
    bir = json.loads(nc.to_json_bytes())
    ctr = 0
    for fn in bir["functions"]:
        for blk in fn["blocks"]:
            out = []
            for inst in blk["instructions"]:
                si = inst.get("sync_info")
                waits = (si or {}).get("on_wait") or []
                if len(waits) > 1:
                    for w in waits[:-1]:
                        ctr += 1
                        out.append(
                            {
                                "debug": inst.get("debug", 0),
                                "engine": inst["engine"],
                                "ins": [],
                                "name": f"{inst['name']}-mw{ctr}",
                                "opcode": "Drain",
                                "outs": [],
                                "sync_info": {"on_update": [], "on_wait": [w]},
                            }
                        )
                    si["on_wait"] = [waits[-1]]
                out.append(inst)
            blk["instructions"] = out
    fixed = json.dumps(bir).encode()
    nc.to_json_bytes = lambda: fixed
    return nc


def _act_reciprocal(nc, out, in_, tmp):
    """1/x on the Activation engine as exp(-ln(x)) — two ACT passes instead
    of one, but Ln and Exp live together in the natural_log_exp_and_others
    activation-table set (and Copy is in every set), so the softmax's exps,
    these reciprocals, and the PSUM->SBUF cast copies never force a ~1.28us
    ACT_TABLE_LOAD: the hardware Reciprocal function lives in a different
    set and was costing 15 table loads per kernel plus a thrash-induced PE
    stall at every q-tile boundary. Denominators are in [1, ~2.5e3], well
    inside both splines' accurate range."""
    nc.scalar.activation(tmp, in_, mybir.ActivationFunctionType.Ln)
    nc.scalar.activation(out, tmp, mybir.ActivationFunctionType.Exp, scale=-1.0)


def build_core_kernel(s=S, d=D, hq=HQ):
    """Bass module for one core: hq query heads + 1 kv head."""
    nst = s // ST  # s-tiles of 512
    ndk = d // 128  # contraction chunks
    nh = hq + 2  # q heads + k + v
    nnt = d // ST  # output n-tiles

    nqk = hq + 1  # q heads + k (fp8 path)

    nq = 4  # quarters per s-tile
    nquar = nst * nq  # 16 global quarter indices
    ndkq = ndk // nq  # bf16 contraction chunks per quarter (V)
    npair = ndk // 2  # 256-row contraction pair-chunks (DoubleRow)
    npq = npair // nq  # fp8 pair-chunks per quarter (QK)

    nc = bass.Bass()
    # host-pre-swizzled: each [k] / [g] slice is per-partition contiguous
    xT_d = nc.dram_tensor("xT", [nquar * 128, ndkq * ST], BF16, kind="ExternalInput")
    xT8_d = nc.dram_tensor("xT8", [nquar * 128, npq * 2 * ST], FP8, kind="ExternalInput")
    wqk8_d = nc.dram_tensor("wqk8", [(npair // 4) * 128, 4 * 2 * nqk * 128], FP8, kind="ExternalInput")
    wvT_d = nc.dram_tensor("wvT", [(ndk // 8) * 128, 8 * 128], BF16, kind="ExternalInput")
    woT_d = nc.dram_tensor("woT", [hq * 128, d], BF16, kind="ExternalInput")
    cosF_d = nc.dram_tensor("cosF", [128, s], BF16, kind="ExternalInput")
    sinSg_d = nc.dram_tensor("sinSg", [128, s], BF16, kind="ExternalInput")
    maskT_d = nc.dram_tensor("maskT", [128, 128], BF16, kind="ExternalInput")
    ident_d = nc.dram_tensor("ident", [128, 128], BF16, kind="ExternalInput")
    onesr_d = nc.dram_tensor("onesr", [128, 128], BF16, kind="ExternalInput")
    outp_d = nc.dram_tensor("outp", [s, d], BF16, kind="ExternalOutput")

    x8r = xT8_d.rearrange("(k p) (i ko n) -> k p i ko n", p=128, i=npq, ko=2)
    xtr = xT_d.rearrange("(k p) (dk n) -> k p dk n", p=128, dk=ndkq)
    wqk8_r = wqk8_d.rearrange("(g p) (j ko m) -> g p j ko m", p=128, j=4, ko=2)
    wv_r = wvT_d.rearrange("(g p) (c n) -> g p c n", p=128, c=8)

    with tile.TileContext(nc) as tc:
        with (
            tc.tile_pool(name="const", bufs=1) as cp,
            tc.tile_pool(name="qkvsb", bufs=1) as qp,
            tc.tile_pool(name="xp8", bufs=5) as xp8,
            tc.tile_pool(name="xpb", bufs=3) as xpb,
            tc.tile_pool(name="rp", bufs=2) as rp,
            tc.tile_pool(name="vp", bufs=4) as vp,
            tc.tile_pool(name="ep", bufs=15) as ep,
            tc.tile_pool(name="dp", bufs=4) as dpool,
            tc.tile_pool(name="tp", bufs=5) as tpool,
            tc.tile_pool(name="sp", bufs=4) as sp,
            tc.tile_pool(name="op", bufs=2) as op,
            tc.tile_pool(name="at", bufs=8) as atp,
        ):
            # ---- resident constants ----
            # per-chunk weight tiles so the first matmul starts after the
            # first small DMA, not after the whole 10MB weight load
            w8g = [
                cp.tile([128, 4, 2, nqk * 128], FP8, tag=f"w8{g}", name=f"w8{g}")
                for g in range(npair // 4)
            ]
            w8 = [w8g[j // 4][:, j % 4, :, :] for j in range(npair)]
            nwg = ndk // 8  # V weight groups of 8 contraction chunks
            wvsb4 = [
                cp.tile([128, 8, 128], BF16, tag=f"wv{g}", name=f"wv{g}")
                for g in range(nwg)
            ]
            # interleave QK and V weight groups on the async SWDGE (gpsimd)
            # queue: sync-queue (HWDGE) dispatches serialize for the whole
            # transfer, so weights there would starve the x-tile loads
            # w8 group 0 is split so the first matmul's pair-chunk lands
            # after ~150KB instead of ~650KB; its first slice rides the
            # sync queue ahead of x8 — the SWDGE path's first-byte latency
            # is ~2us worse and this pair gates the very first matmul
            nc.sync.dma_start(w8g[0][:, 0:1], wqk8_r[0][:, 0:1])
            nc.gpsimd.dma_start(w8g[0][:, 1:4], wqk8_r[0][:, 1:4])
            nc.gpsimd.dma_start(wvsb4[0], wv_r[0])
            for g in range(1, npair // 4):
                nc.gpsimd.dma_start(w8g[g], wqk8_r[g])
                nc.gpsimd.dma_start(wvsb4[g], wv_r[g])
            wvsb = [wvsb4[dk // 8][:, dk % 8, :] for dk in range(ndk)]
            # everything below is loaded on the SYNC queue, interleaved by
            # hand behind the x tiles it must not starve
            cossb = cp.tile([128, s], BF16)
            sinsb = cp.tile([128, s], BF16)
            masksb = cp.tile([128, 128], BF16)
            identsb = cp.tile([128, 128], BF16)
            onesrsb = cp.tile([128, 128], BF16)
            # wo weight loads are spread across the s-tile loop below: they
            # are only needed in phase B and would otherwise crowd the DMA
            # fabric while the first x tiles load
            wosb = [
                cp.tile([128, d], BF16, tag=f"wo{mh}", name=f"wo{mh}")
                for mh in range(hq)
            ]

            # ---- persistent activations (bf16) ----
            qt_sb = [
                qp.tile([128, s], BF16, tag=f"QT{h}", name=f"QT{h}")
                for h in range(hq)
            ]
            kt_sb = qp.tile([128, s], BF16, tag="KT")
            v_sb = qp.tile([128, s], BF16, tag="V")  # [s%128 part, (s//128)*HD]

            # ---- x tile loads: issued several quarters ahead of compute ----
            x8_tiles = {}
            xt_tiles = {}
            cur = {"x8": 0, "xt": 0}

            def issue_x8():
                k = cur["x8"]
                cur["x8"] += 1
                t = xp8.tile([128, npq, 2, ST], FP8, tag="x8", name=f"x8_{k}")
                if k == 0:
                    # split so the first matmul's x lands after 128KB
                    nc.sync.dma_start(t[:, 0:1], x8r[k][:, 0:1])
                    nc.sync.dma_start(t[:, 1:npq], x8r[k][:, 1:npq])
                else:
                    nc.sync.dma_start(t, x8r[k])
                x8_tiles[k] = t

            def issue_xt():
                k = cur["xt"]
                cur["xt"] += 1
                t = xpb.tile([128, ndkq, ST], BF16, tag="xT", name=f"xt_{k}")
                nc.sync.dma_start(t, xtr[k])
                xt_tiles[k] = t

            def pump(x8_upto, xt_upto):
                # issue in need order (smallest quarter index first, x8
                # before xt at ties): a ring-slot wait on a deep-lookahead
                # x8 dispatch then only delays even-less-urgent dispatches
                x8_upto = min(x8_upto, nquar - 1)
                xt_upto = min(xt_upto, nquar - 1)
                while cur["x8"] <= x8_upto or cur["xt"] <= xt_upto:
                    if cur["xt"] <= xt_upto and cur["xt"] < cur["x8"]:
                        issue_xt()
                    elif cur["x8"] <= x8_upto:
                        issue_x8()
                    else:
                        issue_xt()

            # hand-tuned head of the sync queue: fp8 x first (QK path),
            # small consts woven in where first needed
            # constants ride the scalar HWDGE queue (idle until ~27us, and
            # the sync queue's ~8-outstanding-DMA window is precious for x)
            nc.scalar.dma_start(identsb, ident_d[:])
            nc.scalar.dma_start(cossb, cosF_d[:])
            nc.scalar.dma_start(sinsb, sinSg_d[:])
            nc.scalar.dma_start(masksb, maskT_d[:])
            nc.scalar.dma_start(onesrsb, onesr_d[:])
            issue_x8()  # k=0 (split)
            issue_x8()  # k=1
            issue_xt()  # k=0
            issue_x8()  # k=2
            issue_xt()  # k=1
            issue_x8()  # k=3
            issue_xt()  # k=2

            # ================= phase A: QKV projection + RoPE =================
            # e_pre holds exp'd score tiles for q-tile 0, computed during
            # phase A (its K/V/Q deps are all s-tile 0) so phase B can start
            # with attnV immediately
            e_pre = {}
            recips0 = {}
            with (
                tc.tile_pool(name="psA", bufs=6, space="PSUM") as psA,
                tc.tile_pool(name="psE", bufs=1, space="PSUM") as psE,
                tc.tile_pool(name="psT", bufs=1, space="PSUM") as psT,
            ):
                # PE clock warmup: the HAM clock gate defaults to 1.2 GHz and
                # only ramps to 2.4 after ~3.4us of sustained matmul activity.
                # Burn the dead time while the first x DMA is in flight on
                # throwaway matmuls so the first real matmuls run warm. The
                # scratch PSUM tile comes from psE (not psA) so the acc ring
                # stays aligned with phase B's bank-alias assumptions.
                warm = cp.tile([128, ST], BF16, tag="warm")
                nc.vector.memset(warm, 1.0)
                warm_ps = psE.tile([128, ST], F32, tag="sce", name="warmps")
                for i in range(18):
                    nc.tensor.matmul(
                        warm_ps, warm[:, 0:128], warm, start=True, stop=True
                    )

                den0 = {}

                def prefetch_attn0_pair(c, h):
                    # scores + exp + mask + den accumulation for q-tile 0,
                    # chunk c (all diagonal), one head. psE is a 1-deep ring
                    # so the next pair's matmul waits on this exp — callers
                    # space the pairs out between QKV head-groups so the
                    # in-order PE queue never stalls on that wait.
                    off = 128 * c
                    w = ST - off
                    sc_ps = psE.tile([128, ST], F32, tag="sce", name=f"sce{c}_{h}")
                    nc.tensor.matmul(
                        sc_ps[:, 0:w],
                        kt_sb[:, c * 128 : (c + 1) * 128],
                        qt_sb[h][:, off:ST],
                        start=True,
                        stop=True,
                    )
                    e_t = ep.tile([128, ST], BF16, tag="E", name=f"e0_{c}_{h}")
                    nc.scalar.activation(
                        e_t[:, 0:w],
                        sc_ps[:, 0:w],
                        mybir.ActivationFunctionType.Exp,
                        scale=1.0 / HD,
                    )
                    nc.vector.tensor_mul(e_t[:, 0:128], e_t[:, 0:128], masksb)
                    e_pre[(c, h)] = e_t
                    if c == 0:
                        den0[h] = dpool.tile(
                            [128, ST], BF16, tag="den", name=f"den0_{h}"
                        )
                        nc.vector.tensor_copy(den0[h], e_t)
                    else:
                        nc.vector.tensor_add(
                            den0[h][:, off:ST], den0[h][:, off:ST], e_t[:, 0:w]
                        )

                pre_queue = [(c, h) for c in range(4) for h in range(hq)]

                def rope_head(acc, h, ssl, swap_dve=False):
                    # RoPE for one head; write bf16. The half-swap copies run
                    # on the scalar engine (partition-shifted copies are legal
                    # there) to cut the DVE chain to 3 ops per head — except
                    # the first head of each s-tile, whose swap goes to DVE:
                    # its acc bank gates the NEXT s-tile's first matmuls and
                    # the scalar engine is jammed with transpose copies then.
                    dst = qt_sb[h] if h < hq else kt_sb
                    t1 = rp.tile([128, ST], BF16, tag="t1")
                    nc.vector.tensor_mul(t1, acc[h], cossb[:, ssl])
                    tsw = rp.tile([128, ST], BF16, tag="tsw")
                    eng = nc.vector if swap_dve else nc.scalar
                    if swap_dve:
                        nc.vector.tensor_copy(tsw[0:64, :], acc[h][64:128, :])
                        nc.vector.tensor_copy(tsw[64:128, :], acc[h][0:64, :])
                    else:
                        nc.scalar.copy(tsw[0:64, :], acc[h][64:128, :])
                        nc.scalar.copy(tsw[64:128, :], acc[h][0:64, :])
                    nc.vector.tensor_mul(tsw, tsw, sinsb[:, ssl])
                    nc.vector.tensor_add(dst[:, ssl], t1, tsw)

                for st in range(nst):
                    ssl = slice(st * ST, (st + 1) * ST)
                    acc = [
                        psA.tile([128, ST], F32, tag="acc", name=f"acc{h}")
                        for h in range(nh)
                    ]

                    def qk_head_quar(h, quar, x8a):
                        for i in range(npq):
                            nc.tensor.matmul(
                                acc[h],
                                w8[quar * npq + i][:, :, h * 128 : (h + 1) * 128],
                                x8a[:, i, :, :],
                                start=(quar == 0 and i == 0),
                                stop=(quar == nq - 1 and i == npq - 1),
                                perf_mode=mybir.MatmulPerfMode.DoubleRow,
                            )

                    def v_mms(quar, xta):
                        for dk in range(ndkq):
                            nc.tensor.matmul(
                                acc[nh - 1],
                                wvsb[quar * ndkq + dk],
                                xta[:, dk, :],
                                start=(quar == 0 and dk == 0),
                                stop=(quar == nq - 1 and dk == ndkq - 1),
                            )

                    def transpose_v(st):
                        # V: transpose [HD, s-tile] -> [s-chunk, HD] blocks.
                        # All copies on the scalar engine so the transpose
                        # chain (and everything behind it on the in-order PE
                        # queue) doesn't stall on the DVE RoPE backlog.
                        for j in range(ST // 128):
                            vtmp = vp.tile([128, 128], BF16, tag="vtmp")
                            nc.scalar.copy(
                                vtmp, acc[hq + 1][:, j * 128 : (j + 1) * 128]
                            )
                            tp_ps = psT.tile([128, 128], BF16, tag="tp")
                            nc.tensor.transpose(tp_ps, vtmp, identsb)
                            sc = st * (ST // 128) + j
                            nc.scalar.copy(
                                v_sb[:, sc * 128 : (sc + 1) * 128], tp_ps
                            )

                    if st < nst - 1:
                        for quar in range(nq):
                            k = st * nq + quar
                            pump(k + 4, k + 1)
                            # QK for this quarter, with the q-tile 0 attn
                            # prefetch pairs spaced between head-groups of
                            # s-tile 2; V runs one quarter STAGGERED so its
                            # xt tile and wv weights get an extra quarter of
                            # DMA slack (matters most in s-tile 0)
                            if st == 0:
                                # pair-chunk-major: each weight pair-chunk
                                # feeds 5 matmuls as soon as it lands, so
                                # the PE trickles along with the weight DMA
                                # instead of stalling per head
                                for i in range(npq):
                                    for h in range(nqk):
                                        nc.tensor.matmul(
                                            acc[h],
                                            w8[quar * npq + i][:, :, h * 128 : (h + 1) * 128],
                                            x8_tiles[k][:, i, :, :],
                                            start=(quar == 0 and i == 0),
                                            stop=(quar == nq - 1 and i == npq - 1),
                                            perf_mode=mybir.MatmulPerfMode.DoubleRow,
                                        )
                            else:
                                for h in range(nqk):
                                    qk_head_quar(h, quar, x8_tiles[k])
                                    if st == 2 and pre_queue:
                                        prefetch_attn0_pair(*pre_queue.pop(0))
                            if quar > 0:
                                v_mms(quar - 1, xt_tiles[k - 1])
                        v_mms(nq - 1, xt_tiles[st * nq + nq - 1])
                        # wo weights behind this s-tile's x loads
                        nc.sync.dma_start(
                            wosb[st], woT_d[st * 128 : (st + 1) * 128, :]
                        )
                        transpose_v(st)
                        for h in range(hq + 1):
                            rope_head(acc, h, ssl, swap_dve=(h == 0))
                    else:
                        # ---- last s-tile: head-major so each head's RoPE
                        # (and the PSUM bank phase B's attnV q-tile 0 reuses)
                        # completes while later heads still stream ----
                        pump(nquar - 1, nquar - 1)
                        nc.sync.dma_start(
                            wosb[st], woT_d[st * 128 : (st + 1) * 128, :]
                        )
                        base = st * nq
                        for h in range(hq):
                            for quar in range(nq):
                                qk_head_quar(h, quar, x8_tiles[base + quar])
                            rope_head(acc, h, ssl)
                        # q-tile 0 denominator broadcast + reciprocal, woven
                        # between the K quarter-groups so the PE never waits
                        # on the 1-deep psE ring (each bc's reciprocal runs
                        # on ACT while the next K quarter streams)
                        for quar in range(nq):
                            qk_head_quar(hq, quar, x8_tiles[base + quar])
                            bc_ps = psE.tile(
                                [128, ST], F32, tag="sce", name=f"bc0_{quar}"
                            )
                            nc.tensor.matmul(
                                bc_ps, onesrsb, den0[quar], start=True, stop=True
                            )
                            recip = sp.tile(
                                [128, ST], F32, tag="recip", name=f"recip0_{quar}"
                            )
                            lntmp = sp.tile(
                                [128, ST], F32, tag="lntmp", bufs=1,
                                name=f"lnt0_{quar}",
                            )
                            _act_reciprocal(nc, recip, bc_ps, lntmp)
                            recips0[quar] = recip
                        for quar in range(nq):
                            v_mms(quar, xt_tiles[base + quar])
                        # vtmp copies before K's RoPE swaps so the PE
                        # transposes (right before phase B) never wait on a
                        # jammed scalar queue
                        vts = []
                        for j in range(ST // 128):
                            vtmp = vp.tile([128, 128], BF16, tag="vtmp")
                            nc.scalar.copy(
                                vtmp, acc[hq + 1][:, j * 128 : (j + 1) * 128]
                            )
                            vts.append(vtmp)
                        rope_head(acc, hq, ssl)
                        for j, vtmp in enumerate(vts):
                            tp_ps = psT.tile([128, 128], BF16, tag="tp")
                            nc.tensor.transpose(tp_ps, vtmp, identsb)
                            sc = st * (ST // 128) + j
                            nc.scalar.copy(
                                v_sb[:, sc * 128 : (sc + 1) * 128], tp_ps
                            )

            # ============ phase B: attention + output projection ============
            # pool order matters: psAt's banks alias phase A's Q-head acc
            # banks (freed as each head's RoPE completes in the head-major
            # last s-tile), so attnV for the prefetched q-tile 0 can start
            # before the K/V epilogue finishes
            with (
                tc.tile_pool(name="psAt", bufs=4, space="PSUM") as psAt,
                tc.tile_pool(name="psS", bufs=2, space="PSUM") as psS,
                tc.tile_pool(name="psW", bufs=2, space="PSUM") as psW,
            ):
                cast_ctr = [0]
                osb_cur = [None]

                def emit_wo_task(qt, attn_tiles, j, nt):
                    # one output tile of wo for q-tile qt: 4 matmuls
                    # (contraction over the 4 heads) + cast. Four consecutive
                    # nt tiles share one [128, 2048] osb buffer flushed by a
                    # single DMA; the very last s-chunk flushes every 2 tiles
                    # to shorten the drain tail.
                    sc = qt * (ST // 128) + j
                    last_sc = sc == nst * (ST // 128) - 1
                    gran = 2 if last_sc else 4
                    o_ps = psW.tile(
                        [128, ST], F32, tag="wops", name=f"wo{qt}_{j}_{nt}"
                    )
                    for mh in range(hq):
                        nc.tensor.matmul(
                            o_ps,
                            attn_tiles[mh][:, j * 128 : (j + 1) * 128],
                            wosb[mh][:, nt * ST : (nt + 1) * ST],
                            start=(mh == 0),
                            stop=(mh == hq - 1),
                        )
                    if nt % gran == 0:
                        osb_cur[0] = op.tile(
                            [128, 4 * ST], BF16, tag="osb",
                            name=f"osb{qt}_{j}_{nt}",
                        )
                    osb = osb_cur[0]
                    # alternate the PSUM->SBUF cast between ACT and DVE
                    cast_ctr[0] += 1
                    if cast_ctr[0] % 2 == 0:
                        nc.scalar.copy(osb[:, (nt % gran) * ST : (nt % gran + 1) * ST], o_ps)
                    else:
                        nc.vector.tensor_copy(
                            osb[:, (nt % gran) * ST : (nt % gran + 1) * ST], o_ps
                        )
                    if nt % gran == gran - 1:
                        nc.sync.dma_start(
                            outp_d[
                                sc * 128 : (sc + 1) * 128,
                                (nt - gran + 1) * ST : (nt + 1) * ST,
                            ],
                            osb[:, 0 : gran * ST],
                        )

                pending = []  # wo tasks of the previous q-tile
                boundary = []  # held-back tasks two q-tiles old: already
                # normalized, so they can fill the PE during the next
                # q-tile's first chunks while everything else waits on ACT
                deferred_fin = []  # prev q-tile's recip+normalize closures
                for qt in range(nst):
                    nk = (qt + 1) * (ST // 128)  # causal: k chunks this q-tile
                    with nc.named_scope(f"attn{qt}"):
                        at_tiles = {
                            h: psAt.tile([128, ST], F32, tag="at", name=f"at{qt}_{h}")
                            for h in range(hq)
                        }
                        # den accumulation state, all off the PE:
                        # binary-counter tree over full chunks (2x-rate DVE
                        # bf16 adds; GpSimd takes one head), merged into a
                        # single running total when the full chunks end, then
                        # diagonal chunks added in place at their offset as
                        # they arrive — so the q-tile boundary only pays one
                        # add + one broadcast matmul + one reciprocal.
                        tree = {h: {} for h in range(hq)}
                        tot = {}
                        attn_tiles = {}
                        fins = []
                        if qt == 0:
                            # q-tile 0 was fully prefetched in phase A; emit
                            # its attnV head-major so head h's chain issues
                            # as soon as RoPE(Qh) frees its aliased PSUM
                            # bank, instead of gating on all four heads
                            for h in range(hq):
                                for c in range(nk):
                                    o2 = 128 * c
                                    nc.tensor.matmul(
                                        at_tiles[h][:, o2:ST],
                                        v_sb[:, c * 128 : (c + 1) * 128],
                                        e_pre[(c, h)][:, 0 : ST - o2],
                                        start=(c == 0),
                                        stop=(c == nk - 1),
                                    )
                        for c in range(0 if qt == 0 else nk):
                            # diagonal chunks: only columns >= 128*r valid
                            r = c - (nk - 4)
                            off = 128 * r if r > 0 else 0
                            w = ST - off
                            last = c == nk - 1
                            # wo matmuls of the previous q-tile fill the PE
                            # while this chunk's exps run on the scalar
                            # engine; emit them BETWEEN the scores matmuls
                            # (the psS ring is 2 deep, so scores h=2 waits on
                            # exp h=0 — fillers keep the in-order PE queue
                            # fed). Gated until the previous q-tile's
                            # deferred normalize closures have all run: the
                            # wo matmuls read the in-place-normalized tiles.
                            fillers = []
                            if boundary and c < 3:
                                take = min(4, len(boundary))
                                fillers += [boundary.pop(0) for _ in range(take)]
                            if pending and not deferred_fin:
                                hold = 12 if qt < nst - 1 else 0
                                n_emit = -(-max(0, len(pending) - hold) // (nk - c))
                                fillers += [pending.pop(0) for _ in range(n_emit)]
                            nf2 = len(fillers) // 2
                            first_emitted = False
                            e_ts = {}
                            for h in range(hq):
                                sc_ps = psS.tile(
                                    [128, ST], F32, tag="sc",
                                    name=f"sc{qt}_{c}_{h}",
                                )
                                nc.tensor.matmul(
                                    sc_ps[:, 0:w],
                                    kt_sb[:, c * 128 : (c + 1) * 128],
                                    qt_sb[h][:, qt * ST + off : (qt + 1) * ST],
                                    start=True,
                                    stop=True,
                                )
                                e_t = ep.tile(
                                    [128, ST], BF16, tag="E",
                                    name=f"e{qt}_{c}_{h}",
                                )
                                nc.scalar.activation(
                                    e_t[:, 0:w],
                                    sc_ps[:, 0:w],
                                    mybir.ActivationFunctionType.Exp,
                                    scale=1.0 / HD,
                                )
                                if r >= 0:
                                    nc.vector.tensor_mul(
                                        e_t[:, 0:128], e_t[:, 0:128], masksb
                                    )
                                e_ts[h] = e_t
                                if h == 1:
                                    first_emitted = True
                                    for t in fillers[:nf2]:
                                        emit_wo_task(*t)
                            for t in (fillers[nf2:] if first_emitted else fillers):
                                emit_wo_task(*t)
                            for h in range(hq):
                                nc.tensor.matmul(
                                    at_tiles[h][:, off:ST],
                                    v_sb[:, c * 128 : (c + 1) * 128],
                                    e_ts[h][:, 0:w],
                                    start=(c == 0),
                                    stop=(c == nk - 1),
                                )
                            # previous q-tile's deferred reciprocal+normalize
                            # closures: two after each of the first chunks
                            if deferred_fin and c < 2:
                                for _ in range(min(2, len(deferred_fin))):
                                    deferred_fin.pop(0)()
                            # ---- denominator accumulation (off the PE) ----
                            for h in range(hq):
                                eng = nc.vector if h < 3 else nc.gpsimd
                                if off == 0:
                                    # full chunk: binary-counter tree insert
                                    carry = e_ts[h]
                                    lvl = 0
                                    while lvl < 3 and tree[h].get(lvl) is not None:
                                        t_new = tpool.tile(
                                            [128, ST], BF16, tag=f"tr{h}",
                                            name=f"tr{qt}_{c}_{h}_{lvl}",
                                        )
                                        eng.tensor_add(t_new, tree[h][lvl], carry)
                                        tree[h][lvl] = None
                                        carry = t_new
                                        lvl += 1
                                    tree[h][lvl] = carry
                                    if c == nk - 4:
                                        # full chunks end here: merge the
                                        # tree into one running total
                                        lvls = [
                                            l for l in sorted(tree[h])
                                            if tree[h][l] is not None
                                        ]
                                        t_tot = tree[h][lvls[0]]
                                        for l in lvls[1:]:
                                            t_new = tpool.tile(
                                                [128, ST], BF16, tag=f"tr{h}",
                                                name=f"tm{qt}_{c}_{h}_{l}",
                                            )
                                            eng.tensor_add(
                                                t_new, t_tot, tree[h][l]
                                            )
                                            t_tot = t_new
                                        tot[h] = t_tot
                                        tree[h] = {}
                                else:
                                    # diagonal chunk: copy+add into a fresh
                                    # tile — an in-place offset add runs ~5x
                                    # slower on the DVE and jams the masks
                                    # queued behind it
                                    t_new = tpool.tile(
                                        [128, ST], BF16, tag=f"tr{h}",
                                        name=f"td{qt}_{c}_{h}",
                                    )
                                    eng.tensor_copy(
                                        t_new[:, 0:off], tot[h][:, 0:off]
                                    )
                                    eng.tensor_add(
                                        t_new[:, off:ST],
                                        tot[h][:, off:ST],
                                        e_ts[h][:, 0:w],
                                    )
                                    tot[h] = t_new
                                if last:
                                    # free the attnV PSUM bank NOW with a
                                    # plain bf16 copy (no reciprocal dep),
                                    # then defer reduce+reciprocal+in-place
                                    # normalize into the NEXT q-tile's first
                                    # chunks, where ACT interleaves them
                                    # with that q-tile's exps instead of
                                    # serializing at the boundary
                                    atn = atp.tile(
                                        [128, ST], BF16, tag="attnT",
                                        name=f"atn{qt}_{h}",
                                    )
                                    nc.vector.tensor_copy(atn, at_tiles[h])
                                    attn_tiles[h] = atn

                                    def make_fin(tot_h, atn_h, qi, hh):
                                        def fin():
                                            bc_ps = psW.tile(
                                                [128, ST], F32, tag="wops",
                                                name=f"bc{qi}_{hh}",
                                            )
                                            nc.tensor.matmul(
                                                bc_ps, onesrsb, tot_h,
                                                start=True, stop=True,
                                            )
                                            recip = sp.tile(
                                                [128, ST], F32, tag="recip",
                                                name=f"recip{qi}_{hh}",
                                            )
                                            lntmp = sp.tile(
                                                [128, ST], F32, tag="lntmp",
                                                bufs=1, name=f"lnt{qi}_{hh}",
                                            )
                                            _act_reciprocal(
                                                nc, recip, bc_ps, lntmp
                                            )
                                            nc.vector.tensor_mul(
                                                atn_h, atn_h, recip
                                            )
                                        return fin

                                    fins.append(make_fin(tot[h], atn, qt, h))
                        boundary += pending  # <=8 kept-back tasks
                        pending = []
                        if qt == 0:
                            # q-tile 0's reciprocals were computed in phase
                            # A: copy + normalize in place right away
                            for h in range(hq):
                                atn = atp.tile(
                                    [128, ST], BF16, tag="attnT",
                                    name=f"atn0_{h}",
                                )
                                nc.vector.tensor_copy(atn, at_tiles[h])
                                nc.vector.tensor_mul(atn, atn, recips0[h])
                                attn_tiles[h] = atn
                        if qt == nst - 1:
                            # no next q-tile to defer into
                            for fin in fins:
                                fin()
                            fins = []
                        deferred_fin = fins
                    pending = [
                        (qt, attn_tiles, j, nt)
                        for j in range(ST // 128)
                        for nt in range(nnt)
                    ]
                for t in boundary + pending:
                    emit_wo_task(*t)
    return _legalize_single_wait(nc)


def host_prep(x, wq, wk, wv, wo, s=S, d=D, hq=HQ, ncores=NCORES):
    """Shared tensors + per-core weight shards, all host-side numpy.

    Every big tensor is pre-swizzled into exactly the SBUF tile layout the
    kernel loads, so each dma_start moves a per-partition-contiguous block:
      xT8  [16*128, 4096]: row k*128+p, col i*1024+ko*512+n
                           <- x[d=quar*1024+i*256+ko*128+p, s=st*512+n]
      xT   [16*128, 4096]: col dk*512+n <- x[d=quar*1024+dk*128+p, ...]
      wqk8 [4*128, 5120]:  row g*128+p, col j*1280+ko*640+m
                           <- w[d=g*1024+j*256+ko*128+p, m]
      wvT  [4*128, 1024]:  col c*128+n <- w[d=g*1024+c*128+p, n]
    """
    scale = attn_scale(s, HD, MULT)
    xTf = np.ascontiguousarray(x.reshape(s, d).T)
    xT = xTf.astype(NPBF16)
    xT8 = xTf.astype(NPFP8)

    # (quar, i, ko, p, st, n) -> (st, quar, p, i, ko, n)
    x8L = np.ascontiguousarray(
        xT8.reshape(4, 4, 2, 128, 4, 512).transpose(4, 0, 3, 1, 2, 5)
    ).reshape(16 * 128, 4096)
    # (quar, dk, p, st, n) -> (st, quar, p, dk, n)
    xTL = np.ascontiguousarray(
        xT.reshape(4, 8, 128, 4, 512).transpose(3, 0, 2, 1, 4)
    ).reshape(16 * 128, 4096)

    freq = ROPE_BASE ** (-(np.arange(0, HD, 2, dtype=np.float64) / HD))
    pos = np.arange(s, dtype=np.float64)
    angle = pos[:, None] * freq[None, :]  # [s, 64]
    cos = np.cos(angle).astype(NPBF16).T  # [64, s]
    sin = np.sin(angle).astype(NPBF16).T
    cosF = np.ascontiguousarray(np.concatenate([cos, cos], axis=0))
    sinSg = np.ascontiguousarray(np.concatenate([-sin, sin], axis=0))

    # triangular causal mask for diagonal chunks: keep iff p <= f
    p = np.arange(128)[:, None]
    f = np.arange(128)[None, :]
    maskT = (p <= f).astype(NPBF16)  # [128, 128]

    ident = np.eye(128, dtype=NPBF16)
    onesr = np.ones((128, 128), dtype=NPBF16)

    shared = dict(
        xT=xTL, xT8=x8L, cosF=cosF, sinSg=sinSg, maskT=maskT, ident=ident,
        onesr=onesr,
    )

    in_maps = []
    for c in range(ncores):
        wq_c = wq[c * hq * 128 : (c + 1) * hq * 128, :]  # [hq*128, d]
        wk_c = wk[c * 128 : (c + 1) * 128, :]
        wv_c = wv[c * 128 : (c + 1) * 128, :] * scale
        wqk8 = np.ascontiguousarray(
            np.concatenate([wq_c.T, wk_c.T], axis=1)
        ).astype(NPFP8)  # [d, (hq+1)*128]
        # (g, j, ko, p, m) -> (g, p, j, ko, m)
        w8L = np.ascontiguousarray(
            wqk8.reshape(4, 4, 2, 128, 640).transpose(0, 3, 1, 2, 4)
        ).reshape(4 * 128, 5120)
        wvT = np.ascontiguousarray(wv_c.T).astype(NPBF16)  # [d, 128]
        # (g, c, p, n) -> (g, p, c, n)
        wvL = np.ascontiguousarray(
            wvT.reshape(4, 8, 128, 128).transpose(0, 2, 1, 3)
        ).reshape(4 * 128, 1024)
        wo_c = wo[:, c * hq * 128 : (c + 1) * hq * 128]  # [d, hq*128]
        woT = np.ascontiguousarray(wo_c.T).astype(NPBF16)  # [hq*128, d]
        in_maps.append(dict(shared, wqk8=w8L, wvT=wvL, woT=woT))
    return in_maps


_NC_CACHE = {}


def kernel(x, freqs_cis, wq, wk, wv, wo):
    del freqs_cis  # forward pass recomputes rope tables (matches reference)
    x = np.asarray(x, dtype=np.float32)
    key = (S, D, HQ)
    if key not in _NC_CACHE:
        _NC_CACHE[key] = build_core_kernel(S, D, HQ)
    nc = _NC_CACHE[key]
    in_maps = host_prep(
        x, np.asarray(wq, np.float32), np.asarray(wk, np.float32),
        np.asarray(wv, np.float32), np.asarray(wo, np.float32),
    )
    res = run_bass_kernel_spmd(nc, in_maps, core_ids=list(range(NCORES)))
    out = np.zeros((S, D), dtype=np.float32)
    for r in res.results:
        out += np.asarray(r["outp"], dtype=np.float32)
    return out.reshape(B, S, D)


if __name__ == "__main__":
    rng = np.random.default_rng(0)
    x = rng.standard_normal((B, S, D)).astype(np.float32)
    wq = (rng.standard_normal((H * HD, D)) * D**-0.5).astype(np.float32)
    wk = (rng.standard_normal((KVH * HD, D)) * D**-0.5).astype(np.float32)
    wv = (rng.standard_normal((KVH * HD, D)) * D**-0.5).astype(np.float32)
    wo = (rng.standard_normal((D, H * HD)) * (H * HD) ** -0.5).astype(np.float32)
    fc = rng.standard_normal((S, HD // 2)).astype(np.float32)
    out = kernel(x, fc, wq, wk, wv, wo)
    print(out.shape, out.dtype, np.abs(out).max())


# revision 44
# speedup vs baseline: 1.0034x; 1.0034x over previous
"""GQA attention layer (B=1, S=2048, D=4096, H=32, KVH=8, HD=128) on 8 TRN2
NeuronCores, tensor-parallel over heads.

Each core computes 4 query heads + their shared kv head end-to-end:
QKV projection -> RoPE -> causal attention (no-max-sub softmax, scores are
tiny) -> its slice of the wo projection. The 8 partial [S, D] outputs are
summed on the host (the "all-reduce after wo" of the sharding hint).

Device layouts (everything bf16 into the PE, fp32 PSUM accumulation):
  QT/KT  [HD=128(part), S]    from  lhsT=w[d,:], rhs=xT[d, s-tile]
  V      [S(part), HD]        via PE-transpose of VT
  scoresT[k(part), q]         lhsT=KT chunk, rhs=QT tile
  E = exp(scoresT/128) bf16; causal diagonal via 0/1 mask multiply
  attnT  [HD(part), q]        lhsT=V chunk, rhs=E  (accumulated over k)
  den    [128, q] bf16 SBUF   accumulated on DVE/GpSimd (off PE)
  den reduce+broadcast        one matmul lhsT=ones[128,128], rhs=den
  attnT_norm = attnT * recip  (DVE mul, bf16 out)
  out    [s(part), n]         lhsT=attnT_norm chunk, rhs=woT

All DMA-heavy tensors are pre-swizzled on the host into exactly the SBUF
tile layout (per-partition contiguous), so every dma_start is a plain
[128, contiguous] block: cheap to dispatch and line-rate to transfer.
x tile loads run several quarters ahead of compute on the sync queue.

The last s-tile runs its QK matmuls head-major so each head's RoPE (and the
PSUM bank it frees for phase B's prefetched q-tile-0 attnV) completes while
later heads' matmuls still stream; q-tile 0's softmax denominators +
reciprocals are also emitted inside the phase-A tail, hiding the ACT
reciprocal table switch.

wo matmuls for q-tile t-1 are interleaved between the scores and attnV
matmuls of q-tile t so the PE fills the exp-wait gaps (the scalar engine's
exps per chunk exceed the attention matmul time per chunk).
"""

import json
import math

import ml_dtypes
import numpy as np

import concourse.bass as bass
import concourse.tile as tile
from concourse import mybir
from concourse.bass_utils import run_bass_kernel_spmd

BF16 = mybir.dt.bfloat16
F32 = mybir.dt.float32
FP8 = mybir.dt.float8e4
NPBF16 = ml_dtypes.bfloat16
NPFP8 = ml_dtypes.float8_e4m3

# Full problem constants
B, S, D = 1, 2048, 4096
H, KVH = 32, 8
HD = 128
NCORES = 8
HQ = H // NCORES  # query heads per core
MULT = 1.0
ROPE_BASE = 10000.0
ST = 512  # s-tile (PSUM bank width in fp32)


def attn_scale(seq_len=S, d_head=HD, mult=MULT):
    alpha = 1.0 / (1.0 + 4.0 * d_head / mult**2)
    lower = (math.log(seq_len) / seq_len) ** 0.5
    interp = math.exp((1.0 - alpha) * math.log(lower))
    return 1.0 / interp


def _legalize_single_wait(nc):
    """The walrus build in this container accepts only ONE sync wait per
    instruction ("Too many sync wait commands" in setupSyncWait). Split
    extra waits into preceding single-wait Drains (lowered to CTRL NOPs)
    on the same engine — same in-order stall semantics."""
    bir = json.loads(nc.to_json_bytes())
    ctr = 0
    for fn in bir["functions"]:
        for blk in fn["blocks"]:
            out = []
            for inst in blk["instructions"]:
                si = inst.get("sync_info")
                waits = (si or {}).get("on_wait") or []
                if len(waits) > 1:
                    for w in waits[:-1]:
                        ctr += 1
                        out.append(
                            {
                                "debug": inst.get("debug", 0),
                                "engine": inst["engine"],
                                "ins": [],
                                "name": f"{inst['name']}-mw{ctr}",
                                "opcode": "Drain",
                                "outs": [],
                                "sync_info": {"on_update": [], "on_wait": [w]},
                            }
                        )
                    si["on_wait"] = [waits[-1]]
                out.append(inst)
            blk["instructions"] = out
    fixed = json.dumps(bir).encode()
    nc.to_json_bytes = lambda: fixed
    return nc


def _act_reciprocal(nc, out, in_, tmp):
    """1/x on the Activation engine as exp(-ln(x)) — two ACT passes instead
    of one, but Ln and Exp live together in the natural_log_exp_and_others
    activation-table set (and Copy is in every set), so the softmax's exps,
    these reciprocals, and the PSUM->SBUF cast copies never force a ~1.28us
    ACT_TABLE_LOAD: the hardware Reciprocal function lives in a different
    set and was costing 15 table loads per kernel plus a thrash-induced PE
    stall at every q-tile boundary. Denominators are in [1, ~2.5e3], well
    inside both splines' accurate range."""
    nc.scalar.activation(tmp, in_, mybir.ActivationFunctionType.Ln)
    nc.scalar.activation(out, tmp, mybir.ActivationFunctionType.Exp, scale=-1.0)


def build_core_kernel(s=S, d=D, hq=HQ):
    """Bass module for one core: hq query heads + 1 kv head."""
    nst = s // ST  # s-tiles of 512
    ndk = d // 128  # contraction chunks
    nh = hq + 2  # q heads + k + v
    nnt = d // ST  # output n-tiles

    nqk = hq + 1  # q heads + k (fp8 path)

    nq = 4  # quarters per s-tile
    nquar = nst * nq  # 16 global quarter indices
    ndkq = ndk // nq  # bf16 contraction chunks per quarter (V)
    npair = ndk // 2  # 256-row contraction pair-chunks (DoubleRow)
    npq = npair // nq  # fp8 pair-chunks per quarter (QK)

    nc = bass.Bass()
    # host-pre-swizzled: each [k] / [g] slice is per-partition contiguous
    xT_d = nc.dram_tensor("xT", [nquar * 128, ndkq * ST], BF16, kind="ExternalInput")
    xT8_d = nc.dram_tensor("xT8", [nquar * 128, npq * 2 * ST], FP8, kind="ExternalInput")
    wqk8_d = nc.dram_tensor("wqk8", [(npair // 4) * 128, 4 * 2 * nqk * 128], FP8, kind="ExternalInput")
    wvT_d = nc.dram_tensor("wvT", [(ndk // 8) * 128, 8 * 128], BF16, kind="ExternalInput")
    woT_d = nc.dram_tensor("woT", [hq * 128, d], BF16, kind="ExternalInput")
    cosF_d = nc.dram_tensor("cosF", [128, s], BF16, kind="ExternalInput")
    sinSg_d = nc.dram_tensor("sinSg", [128, s], BF16, kind="ExternalInput")
    maskT_d = nc.dram_tensor("maskT", [128, 128], BF16, kind="ExternalInput")
    ident_d = nc.dram_tensor("ident", [128, 128], BF16, kind="ExternalInput")
    onesr_d = nc.dram_tensor("onesr", [128, 128], BF16, kind="ExternalInput")
    outp_d = nc.dram_tensor("outp", [s, d], BF16, kind="ExternalOutput")

    x8r = xT8_d.rearrange("(k p) (i ko n) -> k p i ko n", p=128, i=npq, ko=2)
    xtr = xT_d.rearrange("(k p) (dk n) -> k p dk n", p=128, dk=ndkq)
    wqk8_r = wqk8_d.rearrange("(g p) (j ko m) -> g p j ko m", p=128, j=4, ko=2)
    wv_r = wvT_d.rearrange("(g p) (c n) -> g p c n", p=128, c=8)

    with tile.TileContext(nc) as tc:
        with (
            tc.tile_pool(name="const", bufs=1) as cp,
            tc.tile_pool(name="qkvsb", bufs=1) as qp,
            tc.tile_pool(name="xp8", bufs=5) as xp8,
            tc.tile_pool(name="xpb", bufs=3) as xpb,
            tc.tile_pool(name="rp", bufs=2) as rp,
            tc.tile_pool(name="vp", bufs=4) as vp,
            tc.tile_pool(name="ep", bufs=15) as ep,
            tc.tile_pool(name="dp", bufs=4) as dpool,
            tc.tile_pool(name="tp", bufs=5) as tpool,
            tc.tile_pool(name="sp", bufs=4) as sp,
            tc.tile_pool(name="op", bufs=2) as op,
            tc.tile_pool(name="at", bufs=8) as atp,
        ):
            # ---- resident constants ----
            # per-chunk weight tiles so the first matmul starts after the
            # first small DMA, not after the whole 10MB weight load
            w8g = [
                cp.tile([128, 4, 2, nqk * 128], FP8, tag=f"w8{g}", name=f"w8{g}")
                for g in range(npair // 4)
            ]
            w8 = [w8g[j // 4][:, j % 4, :, :] for j in range(npair)]
            nwg = ndk // 8  # V weight groups of 8 contraction chunks
            wvsb4 = [
                cp.tile([128, 8, 128], BF16, tag=f"wv{g}", name=f"wv{g}")
                for g in range(nwg)
            ]
            # interleave QK and V weight groups on the async SWDGE (gpsimd)
            # queue: sync-queue (HWDGE) dispatches serialize for the whole
            # transfer, so weights there would starve the x-tile loads
            # w8 group 0 is split so the first matmul's pair-chunk lands
            # after ~150KB instead of ~650KB; its first slice rides the
            # sync queue ahead of x8 — the SWDGE path's first-byte latency
            # is ~2us worse and this pair gates the very first matmul
            nc.sync.dma_start(w8g[0][:, 0:1], wqk8_r[0][:, 0:1])
            nc.gpsimd.dma_start(w8g[0][:, 1:4], wqk8_r[0][:, 1:4])
            nc.gpsimd.dma_start(wvsb4[0], wv_r[0])
            for g in range(1, npair // 4):
                nc.gpsimd.dma_start(w8g[g], wqk8_r[g])
                nc.gpsimd.dma_start(wvsb4[g], wv_r[g])
            wvsb = [wvsb4[dk // 8][:, dk % 8, :] for dk in range(ndk)]
            # everything below is loaded on the SYNC queue, interleaved by
            # hand behind the x tiles it must not starve
            cossb = cp.tile([128, s], BF16)
            sinsb = cp.tile([128, s], BF16)
            masksb = cp.tile([128, 128], BF16)
            identsb = cp.tile([128, 128], BF16)
            onesrsb = cp.tile([128, 128], BF16)
            # wo weight loads are spread across the s-tile loop below: they
            # are only needed in phase B and would otherwise crowd the DMA
            # fabric while the first x tiles load
            wosb = [
                cp.tile([128, d], BF16, tag=f"wo{mh}", name=f"wo{mh}")
                for mh in range(hq)
            ]

            # ---- persistent activations (bf16) ----
            qt_sb = [
                qp.tile([128, s], BF16, tag=f"QT{h}", name=f"QT{h}")
                for h in range(hq)
            ]
            kt_sb = qp.tile([128, s], BF16, tag="KT")
            v_sb = qp.tile([128, s], BF16, tag="V")  # [s%128 part, (s//128)*HD]

            # ---- x tile loads: issued several quarters ahead of compute ----
            x8_tiles = {}
            xt_tiles = {}
            cur = {"x8": 0, "xt": 0}

            def issue_x8():
                k = cur["x8"]
                cur["x8"] += 1
                t = xp8.tile([128, npq, 2, ST], FP8, tag="x8", name=f"x8_{k}")
                if k == 0:
                    # split so the first matmul's x lands after 128KB
                    nc.sync.dma_start(t[:, 0:1], x8r[k][:, 0:1])
                    nc.sync.dma_start(t[:, 1:npq], x8r[k][:, 1:npq])
                else:
                    nc.sync.dma_start(t, x8r[k])
                x8_tiles[k] = t

            def issue_xt():
                k = cur["xt"]
                cur["xt"] += 1
                t = xpb.tile([128, ndkq, ST], BF16, tag="xT", name=f"xt_{k}")
                nc.sync.dma_start(t, xtr[k])
                xt_tiles[k] = t

            def pump(x8_upto, xt_upto):
                # issue in need order (smallest quarter index first, x8
                # before xt at ties): a ring-slot wait on a deep-lookahead
                # x8 dispatch then only delays even-less-urgent dispatches
                x8_upto = min(x8_upto, nquar - 1)
                xt_upto = min(xt_upto, nquar - 1)
                while cur["x8"] <= x8_upto or cur["xt"] <= xt_upto:
                    if cur["xt"] <= xt_upto and cur["xt"] < cur["x8"]:
                        issue_xt()
                    elif cur["x8"] <= x8_upto:
                        issue_x8()
                    else:
                        issue_xt()

            # hand-tuned head of the sync queue: fp8 x first (QK path),
            # small consts woven in where first needed
            # constants ride the scalar HWDGE queue (idle until ~27us, and
            # the sync queue's ~8-outstanding-DMA window is precious for x)
            nc.scalar.dma_start(identsb, ident_d[:])
            nc.scalar.dma_start(cossb, cosF_d[:])
            nc.scalar.dma_start(sinsb, sinSg_d[:])
            nc.scalar.dma_start(masksb, maskT_d[:])
            nc.scalar.dma_start(onesrsb, onesr_d[:])
            issue_x8()  # k=0 (split)
            issue_x8()  # k=1
            issue_xt()  # k=0
            issue_x8()  # k=2
            issue_xt()  # k=1
            issue_x8()  # k=3
            issue_xt()  # k=2

            # ================= phase A: QKV projection + RoPE =================
            # e_pre holds exp'd score tiles for q-tile 0, computed during
            # phase A (its K/V/Q deps are all s-tile 0) so phase B can start
            # with attnV immediately
            e_pre = {}
            recips0 = {}
            with (
                tc.tile_pool(name="psA", bufs=6, space="PSUM") as psA,
                tc.tile_pool(name="psE", bufs=1, space="PSUM") as psE,
                tc.tile_pool(name="psT", bufs=1, space="PSUM") as psT,
            ):
                # PE clock warmup: the HAM clock gate defaults to 1.2 GHz and
                # only ramps to 2.4 after ~3.4us of sustained matmul activity.
                # Burn the dead time while the first x DMA is in flight on
                # throwaway matmuls so the first real matmuls run warm. The
                # scratch PSUM tile comes from psE (not psA) so the acc ring
                # stays aligned with phase B's bank-alias assumptions.
                warm = cp.tile([128, ST], BF16, tag="warm")
                nc.vector.memset(warm, 1.0)
                warm_ps = psE.tile([128, ST], F32, tag="sce", name="warmps")
                for i in range(18):
                    nc.tensor.matmul(
                        warm_ps, warm[:, 0:128], warm, start=True, stop=True
                    )

                den0 = {}

                def prefetch_attn0_pair(c, h):
                    # scores + exp + mask + den accumulation for q-tile 0,
                    # chunk c (all diagonal), one head. psE is a 1-deep ring
                    # so the next pair's matmul waits on this exp — callers
                    # space the pairs out between QKV head-groups so the
                    # in-order PE queue never stalls on that wait.
                    off = 128 * c
                    w = ST - off
                    sc_ps = psE.tile([128, ST], F32, tag="sce", name=f"sce{c}_{h}")
                    nc.tensor.matmul(
                        sc_ps[:, 0:w],
                        kt_sb[:, c * 128 : (c + 1) * 128],
                        qt_sb[h][:, off:ST],
                        start=True,
                        stop=True,
                    )
                    e_t = ep.tile([128, ST], BF16, tag="E", name=f"e0_{c}_{h}")
                    nc.scalar.activation(
                        e_t[:, 0:w],
                        sc_ps[:, 0:w],
                        mybir.ActivationFunctionType.Exp,
                        scale=1.0 / HD,
                    )
                    nc.vector.tensor_mul(e_t[:, 0:128], e_t[:, 0:128], masksb)
                    e_pre[(c, h)] = e_t
                    if c == 0:
                        den0[h] = dpool.tile(
                            [128, ST], BF16, tag="den", name=f"den0_{h}"
                        )
                        nc.vector.tensor_copy(den0[h], e_t)
                    else:
                        nc.vector.tensor_add(
                            den0[h][:, off:ST], den0[h][:, off:ST], e_t[:, 0:w]
                        )

                pre_queue = [(c, h) for c in range(4) for h in range(hq)]

                def rope_head(acc, h, ssl, swap_dve=False):
                    # RoPE for one head; write bf16. The half-swap copies run
                    # on the scalar engine (partition-shifted copies are legal
                    # there) to cut the DVE chain to 3 ops per head — except
                    # the first head of each s-tile, whose swap goes to DVE:
                    # its acc bank gates the NEXT s-tile's first matmuls and
                    # the scalar engine is jammed with transpose copies then.
                    dst = qt_sb[h] if h < hq else kt_sb
                    t1 = rp.tile([128, ST], BF16, tag="t1")
                    nc.vector.tensor_mul(t1, acc[h], cossb[:, ssl])
                    tsw = rp.tile([128, ST], BF16, tag="tsw")
                    eng = nc.vector if swap_dve else nc.scalar
                    if swap_dve:
                        nc.vector.tensor_copy(tsw[0:64, :], acc[h][64:128, :])
                        nc.vector.tensor_copy(tsw[64:128, :], acc[h][0:64, :])
                    else:
                        nc.scalar.copy(tsw[0:64, :], acc[h][64:128, :])
                        nc.scalar.copy(tsw[64:128, :], acc[h][0:64, :])
                    nc.vector.tensor_mul(tsw, tsw, sinsb[:, ssl])
                    nc.vector.tensor_add(dst[:, ssl], t1, tsw)

                for st in range(nst):
                    ssl = slice(st * ST, (st + 1) * ST)
                    acc = [
                        psA.tile([128, ST], F32, tag="acc", name=f"acc{h}")
                        for h in range(nh)
                    ]

                    def qk_head_quar(h, quar, x8a):
                        for i in range(npq):
                            nc.tensor.matmul(
                                acc[h],
                                w8[quar * npq + i][:, :, h * 128 : (h + 1) * 128],
                                x8a[:, i, :, :],
                                start=(quar == 0 and i == 0),
                                stop=(quar == nq - 1 and i == npq - 1),
                                perf_mode=mybir.MatmulPerfMode.DoubleRow,
                            )

                    def v_mms(quar, xta):
                        for dk in range(ndkq):
                            nc.tensor.matmul(
                                acc[nh - 1],
                                wvsb[quar * ndkq + dk],
                                xta[:, dk, :],
                                start=(quar == 0 and dk == 0),
                                stop=(quar == nq - 1 and dk == ndkq - 1),
                            )

                    def transpose_v(st):
                        # V: transpose [HD, s-tile] -> [s-chunk, HD] blocks.
                        # All copies on the scalar engine so the transpose
                        # chain (and everything behind it on the in-order PE
                        # queue) doesn't stall on the DVE RoPE backlog.
                        for j in range(ST // 128):
                            vtmp = vp.tile([128, 128], BF16, tag="vtmp")
                            nc.scalar.copy(
                                vtmp, acc[hq + 1][:, j * 128 : (j + 1) * 128]
                            )
                            tp_ps = psT.tile([128, 128], BF16, tag="tp")
                            nc.tensor.transpose(tp_ps, vtmp, identsb)
                            sc = st * (ST // 128) + j
                            nc.scalar.copy(
                                v_sb[:, sc * 128 : (sc + 1) * 128], tp_ps
                            )

                    if st < nst - 1:
                        for quar in range(nq):
                            k = st * nq + quar
                            pump(k + 4, k + 1)
                            # QK for this quarter, with the q-tile 0 attn
                            # prefetch pairs spaced between head-groups of
                            # s-tile 2; V runs one quarter STAGGERED so its
                            # xt tile and wv weights get an extra quarter of
                            # DMA slack (matters most in s-tile 0)
                            if st == 0:
                                # pair-chunk-major: each weight pair-chunk
                                # feeds 5 matmuls as soon as it lands, so
                                # the PE trickles along with the weight DMA
                                # instead of stalling per head
                                for i in range(npq):
                                    for h in range(nqk):
                                        nc.tensor.matmul(
                                            acc[h],
                                            w8[quar * npq + i][:, :, h * 128 : (h + 1) * 128],
                                            x8_tiles[k][:, i, :, :],
                                            start=(quar == 0 and i == 0),
                                            stop=(quar == nq - 1 and i == npq - 1),
                                            perf_mode=mybir.MatmulPerfMode.DoubleRow,
                                        )
                            else:
                                for h in range(nqk):
                                    qk_head_quar(h, quar, x8_tiles[k])
                                    if st == 2 and pre_queue:
                                        prefetch_attn0_pair(*pre_queue.pop(0))
                            if quar > 0:
                                v_mms(quar - 1, xt_tiles[k - 1])
                        v_mms(nq - 1, xt_tiles[st * nq + nq - 1])
                        # wo weights behind this s-tile's x loads
                        nc.sync.dma_start(
                            wosb[st], woT_d[st * 128 : (st + 1) * 128, :]
                        )
                        transpose_v(st)
                        for h in range(hq + 1):
                            rope_head(acc, h, ssl, swap_dve=(h == 0))
                    else:
                        # ---- last s-tile: head-major so each head's RoPE
                        # (and the PSUM bank phase B's attnV q-tile 0 reuses)
                        # completes while later heads still stream ----
                        pump(nquar - 1, nquar - 1)
                        nc.sync.dma_start(
                            wosb[st], woT_d[st * 128 : (st + 1) * 128, :]
                        )
                        base = st * nq
                        for h in range(hq):
                            for quar in range(nq):
                                qk_head_quar(h, quar, x8_tiles[base + quar])
                            rope_head(acc, h, ssl)
                        # q-tile 0 denominator broadcast + reciprocal, woven
                        # between the K quarter-groups so the PE never waits
                        # on the 1-deep psE ring (each bc's reciprocal runs
                        # on ACT while the next K quarter streams)
                        for quar in range(nq):
                            qk_head_quar(hq, quar, x8_tiles[base + quar])
                            bc_ps = psE.tile(
                                [128, ST], F32, tag="sce", name=f"bc0_{quar}"
                            )
                            nc.tensor.matmul(
                                bc_ps, onesrsb, den0[quar], start=True, stop=True
                            )
                            recip = sp.tile(
                                [128, ST], F32, tag="recip", name=f"recip0_{quar}"
                            )
                            lntmp = sp.tile(
                                [128, ST], F32, tag="lntmp", bufs=1,
                                name=f"lnt0_{quar}",
                            )
                            _act_reciprocal(nc, recip, bc_ps, lntmp)
                            recips0[quar] = recip
                        for quar in range(nq):
                            v_mms(quar, xt_tiles[base + quar])
                        # vtmp copies before K's RoPE swaps so the PE
                        # transposes (right before phase B) never wait on a
                        # jammed scalar queue
                        vts = []
                        for j in range(ST // 128):
                            vtmp = vp.tile([128, 128], BF16, tag="vtmp")
                            nc.scalar.copy(
                                vtmp, acc[hq + 1][:, j * 128 : (j + 1) * 128]
                            )
                            vts.append(vtmp)
                        rope_head(acc, hq, ssl)
                        for j, vtmp in enumerate(vts):
                            tp_ps = psT.tile([128, 128], BF16, tag="tp")
                            nc.tensor.transpose(tp_ps, vtmp, identsb)
                            sc = st * (ST // 128) + j
                            nc.scalar.copy(
                                v_sb[:, sc * 128 : (sc + 1) * 128], tp_ps
                            )

            # ============ phase B: attention + output projection ============
            # pool order matters: psAt's banks alias phase A's Q-head acc
            # banks (freed as each head's RoPE completes in the head-major
            # last s-tile), so attnV for the prefetched q-tile 0 can start
            # before the K/V epilogue finishes
            with (
                tc.tile_pool(name="psAt", bufs=4, space="PSUM") as psAt,
                tc.tile_pool(name="psS", bufs=2, space="PSUM") as psS,
                tc.tile_pool(name="psW", bufs=2, space="PSUM") as psW,
            ):
                cast_ctr = [0]
                osb_cur = [None]

                def emit_wo_task(qt, attn_tiles, j, nt):
                    # one output tile of wo for q-tile qt: 4 matmuls
                    # (contraction over the 4 heads) + cast. Four consecutive
                    # nt tiles share one [128, 2048] osb buffer flushed by a
                    # single DMA; the very last s-chunk flushes every 2 tiles
                    # to shorten the drain tail.
                    sc = qt * (ST // 128) + j
                    last_sc = sc == nst * (ST // 128) - 1
                    gran = 2 if last_sc else 4
                    o_ps = psW.tile(
                        [128, ST], F32, tag="wops", name=f"wo{qt}_{j}_{nt}"
                    )
                    for mh in range(hq):
                        nc.tensor.matmul(
                            o_ps,
                            attn_tiles[mh][:, j * 128 : (j + 1) * 128],
                            wosb[mh][:, nt * ST : (nt + 1) * ST],
                            start=(mh == 0),
                            stop=(mh == hq - 1),
                        )
                    if nt % gran == 0:
                        osb_cur[0] = op.tile(
                            [128, 4 * ST], BF16, tag="osb",
                            name=f"osb{qt}_{j}_{nt}",
                        )
                    osb = osb_cur[0]
                    # alternate the PSUM->SBUF cast between ACT and DVE
                    cast_ctr[0] += 1
                    if cast_ctr[0] % 2 == 0:
                        nc.scalar.copy(osb[:, (nt % gran) * ST : (nt % gran + 1) * ST], o_ps)
                    else:
                        nc.vector.tensor_copy(
                            osb[:, (nt % gran) * ST : (nt % gran + 1) * ST], o_ps
                        )
                    if nt % gran == gran - 1:
                        nc.sync.dma_start(
                            outp_d[
                                sc * 128 : (sc + 1) * 128,
                                (nt - gran + 1) * ST : (nt + 1) * ST,
                            ],
                            osb[:, 0 : gran * ST],
                        )

                pending = []  # wo tasks of the previous q-tile
                boundary = []  # held-back tasks two q-tiles old: already
                # normalized, so they can fill the PE during the next
                # q-tile's first chunks while everything else waits on ACT
                deferred_fin = []  # prev q-tile's recip+normalize closures
                for qt in range(nst):
                    nk = (qt + 1) * (ST // 128)  # causal: k chunks this q-tile
                    with nc.named_scope(f"attn{qt}"):
                        at_tiles = {
                            h: psAt.tile([128, ST], F32, tag="at", name=f"at{qt}_{h}")
                            for h in range(hq)
                        }
                        # den accumulation state, all off the PE:
                        # binary-counter tree over full chunks (2x-rate DVE
                        # bf16 adds; GpSimd takes one head), merged into a
                        # single running total when the full chunks end, then
                        # diagonal chunks added in place at their offset as
                        # they arrive — so the q-tile boundary only pays one
                        # add + one broadcast matmul + one reciprocal.
                        tree = {h: {} for h in range(hq)}
                        tot = {}
                        attn_tiles = {}
                        fins = []
                        if qt == 0:
                            # q-tile 0 was fully prefetched in phase A; emit
                            # its attnV head-major so head h's chain issues
                            # as soon as RoPE(Qh) frees its aliased PSUM
                            # bank, instead of gating on all four heads
                            for h in range(hq):
                                for c in range(nk):
                                    o2 = 128 * c
                                    nc.tensor.matmul(
                                        at_tiles[h][:, o2:ST],
                                        v_sb[:, c * 128 : (c + 1) * 128],
                                        e_pre[(c, h)][:, 0 : ST - o2],
                                        start=(c == 0),
                                        stop=(c == nk - 1),
                                    )
                        for c in range(0 if qt == 0 else nk):
                            # diagonal chunks: only columns >= 128*r valid
                            r = c - (nk - 4)
                            off = 128 * r if r > 0 else 0
                            w = ST - off
                            last = c == nk - 1
                            # wo matmuls of the previous q-tile fill the PE
                            # while this chunk's exps run on the scalar
                            # engine; emit them BETWEEN the scores matmuls
                            # (the psS ring is 2 deep, so scores h=2 waits on
                            # exp h=0 — fillers keep the in-order PE queue
                            # fed). Gated until the previous q-tile's
                            # deferred normalize closures have all run: the
                            # wo matmuls read the in-place-normalized tiles.
                            fillers = []
                            if boundary and c < 3:
                                take = min(4, len(boundary))
                                fillers += [boundary.pop(0) for _ in range(take)]
                            if pending and not deferred_fin:
                                hold = 12 if qt < nst - 1 else 0
                                n_emit = -(-max(0, len(pending) - hold) // (nk - c))
                                fillers += [pending.pop(0) for _ in range(n_emit)]
                            nf2 = len(fillers) // 2
                            first_emitted = False
                            e_ts = {}
                            for h in range(hq):
                                sc_ps = psS.tile(
                                    [128, ST], F32, tag="sc",
                                    name=f"sc{qt}_{c}_{h}",
                                )
                                nc.tensor.matmul(
                                    sc_ps[:, 0:w],
                                    kt_sb[:, c * 128 : (c + 1) * 128],
                                    qt_sb[h][:, qt * ST + off : (qt + 1) * ST],
                                    start=True,
                                    stop=True,
                                )
                                e_t = ep.tile(
                                    [128, ST], BF16, tag="E",
                                    name=f"e{qt}_{c}_{h}",
                                )
                                nc.scalar.activation(
                                    e_t[:, 0:w],
                                    sc_ps[:, 0:w],
                                    mybir.ActivationFunctionType.Exp,
                                    scale=1.0 / HD,
                                )
                                if r >= 0:
                                    nc.vector.tensor_mul(
                                        e_t[:, 0:128], e_t[:, 0:128], masksb
                                    )
                                e_ts[h] = e_t
                                if h == 1:
                                    first_emitted = True
                                    for t in fillers[:nf2]:
                                        emit_wo_task(*t)
                            for t in (fillers[nf2:] if first_emitted else fillers):
                                emit_wo_task(*t)
                            for h in range(hq):
                                nc.tensor.matmul(
                                    at_tiles[h][:, off:ST],
                                    v_sb[:, c * 128 : (c + 1) * 128],
                                    e_ts[h][:, 0:w],
                                    start=(c == 0),
                                    stop=(c == nk - 1),
                                )
                            # previous q-tile's deferred reciprocal+normalize
                            # closures: two after each of the first chunks
                            if deferred_fin and c < 2:
                                for _ in range(min(2, len(deferred_fin))):
                                    deferred_fin.pop(0)()
                            # ---- denominator accumulation (off the PE) ----
                            for h in range(hq):
                                eng = nc.vector if h < 3 else nc.gpsimd
                                if off == 0:
                                    # full chunk: binary-counter tree insert
                                    carry = e_ts[h]
                                    lvl = 0
                                    while lvl < 3 and tree[h].get(lvl) is not None:
                                        t_new = tpool.tile(
                                            [128, ST], BF16, tag=f"tr{h}",
                                            name=f"tr{qt}_{c}_{h}_{lvl}",
                                        )
                                        eng.tensor_add(t_new, tree[h][lvl], carry)
                                        tree[h][lvl] = None
                                        carry = t_new
                                        lvl += 1
                                    tree[h][lvl] = carry
                                    if c == nk - 4:
                                        # full chunks end here: merge the
                                        # tree into one running total
                                        lvls = [
                                            l for l in sorted(tree[h])
                                            if tree[h][l] is not None
                                        ]
                                        t_tot = tree[h][lvls[0]]
                                        for l in lvls[1:]:
                                            t_new = tpool.tile(
                                                [128, ST], BF16, tag=f"tr{h}",
                                                name=f"tm{qt}_{c}_{h}_{l}",
                                            )
                                            eng.tensor_add(
                                                t_new, t_tot, tree[h][l]
                                            )
                                            t_tot = t_new
                                        tot[h] = t_tot
                                        tree[h] = {}
                                else:
                                    # diagonal chunk: add in place at offset
                                    eng.tensor_add(
                                        tot[h][:, off:ST],
                                        tot[h][:, off:ST],
                                        e_ts[h][:, 0:w],
                                    )
                                if last:
                                    # free the attnV PSUM bank NOW with a
                                    # plain bf16 copy (no reciprocal dep),
                                    # then defer reduce+reciprocal+in-place
                                    # normalize into the NEXT q-tile's first
                                    # chunks, where ACT interleaves them
                                    # with that q-tile's exps instead of
                                    # serializing at the boundary
                                    atn = atp.tile(
                                        [128, ST], BF16, tag="attnT",
                                        name=f"atn{qt}_{h}",
                                    )
                                    nc.vector.tensor_copy(atn, at_tiles[h])
                                    attn_tiles[h] = atn

                                    def make_fin(tot_h, atn_h, qi, hh):
                                        def fin():
                                            bc_ps = psW.tile(
                                                [128, ST], F32, tag="wops",
                                                name=f"bc{qi}_{hh}",
                                            )
                                            nc.tensor.matmul(
                                                bc_ps, onesrsb, tot_h,
                                                start=True, stop=True,
                                            )
                                            recip = sp.tile(
                                                [128, ST], F32, tag="recip",
                                                name=f"recip{qi}_{hh}",
                                            )
                                            lntmp = sp.tile(
                                                [128, ST], F32, tag="lntmp",
                                                bufs=1, name=f"lnt{qi}_{hh}",
                                            )
                                            _act_reciprocal(
                                                nc, recip, bc_ps, lntmp
                                            )
                                            nc.vector.tensor_mul(
                                                atn_h, atn_h, recip
                                            )
                                        return fin

                                    fins.append(make_fin(tot[h], atn, qt, h))
                        boundary += pending  # <=8 kept-back tasks
                        pending = []
                        if qt == 0:
                            # q-tile 0's reciprocals were computed in phase
                            # A: copy + normalize in place right away
                            for h in range(hq):
                                atn = atp.tile(
                                    [128, ST], BF16, tag="attnT",
                                    name=f"atn0_{h}",
                                )
                                nc.vector.tensor_copy(atn, at_tiles[h])
                                nc.vector.tensor_mul(atn, atn, recips0[h])
                                attn_tiles[h] = atn
                        if qt == nst - 1:
                            # no next q-tile to defer into
                            for fin in fins:
                                fin()
                            fins = []
                        deferred_fin = fins
                    pending = [
                        (qt, attn_tiles, j, nt)
                        for j in range(ST // 128)
                        for nt in range(nnt)
                    ]
                for t in boundary + pending:
                    emit_wo_task(*t)
    return _legalize_single_wait(nc)


def host_prep(x, wq, wk, wv, wo, s=S, d=D, hq=HQ, ncores=NCORES):
    """Shared tensors + per-core weight shards, all host-side numpy.

    Every big tensor is pre-swizzled into exactly the SBUF tile layout the
    kernel loads, so each dma_start moves a per-partition-contiguous block:
      xT8  [16*128, 4096]: row k*128+p, col i*1024+ko*512+n
                           <- x[d=quar*1024+i*256+ko*128+p, s=st*512+n]
      xT   [16*128, 4096]: col dk*512+n <- x[d=quar*1024+dk*128+p, ...]
      wqk8 [4*128, 5120]:  row g*128+p, col j*1280+ko*640+m
                           <- w[d=g*1024+j*256+ko*128+p, m]
      wvT  [4*128, 1024]:  col c*128+n <- w[d=g*1024+c*128+p, n]
    """
    scale = attn_scale(s, HD, MULT)
    xTf = np.ascontiguousarray(x.reshape(s, d).T)
    xT = xTf.astype(NPBF16)
    xT8 = xTf.astype(NPFP8)

    # (quar, i, ko, p, st, n) -> (st, quar, p, i, ko, n)
    x8L = np.ascontiguousarray(
        xT8.reshape(4, 4, 2, 128, 4, 512).transpose(4, 0, 3, 1, 2, 5)
    ).reshape(16 * 128, 4096)
    # (quar, dk, p, st, n) -> (st, quar, p, dk, n)
    xTL = np.ascontiguousarray(
        xT.reshape(4, 8, 128, 4, 512).transpose(3, 0, 2, 1, 4)
    ).reshape(16 * 128, 4096)

    freq = ROPE_BASE ** (-(np.arange(0, HD, 2, dtype=np.float64) / HD))
    pos = np.arange(s, dtype=np.float64)
    angle = pos[:, None] * freq[None, :]  # [s, 64]
    cos = np.cos(angle).astype(NPBF16).T  # [64, s]
    sin = np.sin(angle).astype(NPBF16).T
    cosF = np.ascontiguousarray(np.concatenate([cos, cos], axis=0))
    sinSg = np.ascontiguousarray(np.concatenate([-sin, sin], axis=0))

    # triangular causal mask for diagonal chunks: keep iff p <= f
    p = np.arange(128)[:, None]
    f = np.arange(128)[None, :]
    maskT = (p <= f).astype(NPBF16)  # [128, 128]

    ident = np.eye(128, dtype=NPBF16)
    onesr = np.ones((128, 128), dtype=NPBF16)

    shared = dict(
        xT=xTL, xT8=x8L, cosF=cosF, sinSg=sinSg, maskT=maskT, ident=ident,
        onesr=onesr,
    )

    in_maps = []
    for c in range(ncores):
        wq_c = wq[c * hq * 128 : (c + 1) * hq * 128, :]  # [hq*128, d]
        wk_c = wk[c * 128 : (c + 1) * 128, :]
        wv_c = wv[c * 128 : (c + 1) * 128, :] * scale
        wqk8 = np.ascontiguousarray(
            np.concatenate([wq_c.T, wk_c.T], axis=1)
        ).astype(NPFP8)  # [d, (hq+1)*128]
        # (g, j, ko, p, m) -> (g, p, j, ko, m)
        w8L = np.ascontiguousarray(
            wqk8.reshape(4, 4, 2, 128, 640).transpose(0, 3, 1, 2, 4)
        ).reshape(4 * 128, 5120)
        wvT = np.ascontiguousarray(wv_c.T).astype(NPBF16)  # [d, 128]
        # (g, c, p, n) -> (g, p, c, n)
        wvL = np.ascontiguousarray(
            wvT.reshape(4, 8, 128, 128).transpose(0, 2, 1, 3)
        ).reshape(4 * 128, 1024)
        wo_c = wo[:, c * hq * 128 : (c + 1) * hq * 128]  # [d, hq*128]
        woT = np.ascontiguousarray(wo_c.T).astype(NPBF16)  # [hq*128, d]
        in_maps.append(dict(shared, wqk8=w8L, wvT=wvL, woT=woT))
    return in_maps


_NC_CACHE = {}


def kernel(x, freqs_cis, wq, wk, wv, wo):
    del freqs_cis  # forward pass recomputes rope tables (matches reference)
    x = np.asarray(x, dtype=np.float32)
    key = (S, D, HQ)
    if key not in _NC_CACHE:
        _NC_CACHE[key] = build_core_kernel(S, D, HQ)
    nc = _NC_CACHE[key]
    in_maps = host_prep(
        x, np.asarray(wq, np.float32), np.asarray(wk, np.float32),
        np.asarray(wv, np.float32), np.asarray(wo, np.float32),
    )
    res = run_bass_kernel_spmd(nc, in_maps, core_ids=list(range(NCORES)))
    out = np.zeros((S, D), dtype=np.float32)
    for r in res.results:
        out += np.asarray(r["outp"], dtype=np.float32)
    return out.reshape(B, S, D)


if __name__ == "__main__":
    rng = np.random.default_rng(0)
    x = rng.standard_normal((B, S, D)).astype(np.float32)
    wq = (rng.standard_normal((H * HD, D)) * D**-0.5).astype(np.float32)
    wk = (rng.standard_normal((KVH * HD, D)) * D**-0.5).astype(np.float32)
    wv = (rng.standard_normal((KVH * HD, D)) * D**-0.5).astype(np.float32)
    wo = (rng.standard_normal((D, H * HD)) * (H * HD) ** -0.5).astype(np.float32)
    fc = rng.standard_normal((S, HD // 2)).astype(np.float32)
    out = kernel(x, fc, wq, wk, wv, wo)
    print(out.shape, out.dtype, np.abs(out).max())


# revision 50
# speedup vs baseline: 1.0179x; 1.0144x over previous
"""GQA attention layer (B=1, S=2048, D=4096, H=32, KVH=8, HD=128) on 8 TRN2
NeuronCores, tensor-parallel over heads.

Each core computes 4 query heads + their shared kv head end-to-end:
QKV projection -> RoPE -> causal attention (no-max-sub softmax, scores are
tiny) -> its slice of the wo projection. The 8 partial [S, D] outputs are
summed on the host (the "all-reduce after wo" of the sharding hint).

Device layouts (everything bf16 into the PE, fp32 PSUM accumulation):
  QT/KT  [HD=128(part), S]    from  lhsT=w[d,:], rhs=xT[d, s-tile]
  V      [S(part), HD]        via PE-transpose of VT
  scoresT[k(part), q]         lhsT=KT chunk, rhs=QT tile
  E = exp(scoresT/128) bf16; causal diagonal via 0/1 mask multiply
  attnT  [HD(part), q]        lhsT=V chunk, rhs=E  (accumulated over k)
  den    [128, q] bf16 SBUF   accumulated on DVE/GpSimd (off PE)
  den reduce+broadcast        one matmul lhsT=ones[128,128], rhs=den
  attnT_norm = attnT * recip  (DVE mul, bf16 out)
  out    [s(part), n]         lhsT=attnT_norm chunk, rhs=woT

All DMA-heavy tensors are pre-swizzled on the host into exactly the SBUF
tile layout (per-partition contiguous), so every dma_start is a plain
[128, contiguous] block: cheap to dispatch and line-rate to transfer.
x tile loads run several quarters ahead of compute on the sync queue.

The last s-tile runs its QK matmuls head-major so each head's RoPE (and the
PSUM bank it frees for phase B's prefetched q-tile-0 attnV) completes while
later heads' matmuls still stream; q-tile 0's softmax denominators +
reciprocals are also emitted inside the phase-A tail, hiding the ACT
reciprocal table switch.

wo matmuls for q-tile t-1 are interleaved between the scores and attnV
matmuls of q-tile t so the PE fills the exp-wait gaps (the scalar engine's
exps per chunk exceed the attention matmul time per chunk).
"""

import json
import math

import ml_dtypes
import numpy as np

import concourse.bass as bass
import concourse.tile as tile
from concourse import mybir
from concourse.bass_utils import run_bass_kernel_spmd

BF16 = mybir.dt.bfloat16
F32 = mybir.dt.float32
FP8 = mybir.dt.float8e4
NPBF16 = ml_dtypes.bfloat16
NPFP8 = ml_dtypes.float8_e4m3

# Full problem constants
B, S, D = 1, 2048, 4096
H, KVH = 32, 8
HD = 128
NCORES = 8
HQ = H // NCORES  # query heads per core
MULT = 1.0
ROPE_BASE = 10000.0
ST = 512  # s-tile (PSUM bank width in fp32)


def attn_scale(seq_len=S, d_head=HD, mult=MULT):
    alpha = 1.0 / (1.0 + 4.0 * d_head / mult**2)
    lower = (math.log(seq_len) / seq_len) ** 0.5
    interp = math.exp((1.0 - alpha) * math.log(lower))
    return 1.0 / interp


def _legalize_single_wait(nc):
    """The walrus build in this container accepts only ONE sync wait per
    instruction ("Too many sync wait commands" in setupSyncWait). Split
    extra waits into preceding single-wait Drains (lowered to CTRL NOPs)
    on the same engine — same in-order stall semantics."""
    bir = json.loads(nc.to_json_bytes())
    ctr = 0
    for fn in bir["functions"]:
        for blk in fn["blocks"]:
            out = []
            for inst in blk["instructions"]:
                si = inst.get("sync_info")
                waits = (si or {}).get("on_wait") or []
                if len(waits) > 1:
                    for w in waits[:-1]:
                        ctr += 1
                        out.append(
                            {
                                "debug": inst.get("debug", 0),
                                "engine": inst["engine"],
                                "ins": [],
                                "name": f"{inst['name']}-mw{ctr}",
                                "opcode": "Drain",
                                "outs": [],
                                "sync_info": {"on_update": [], "on_wait": [w]},
                            }
                        )
                    si["on_wait"] = [waits[-1]]
                out.append(inst)
            blk["instructions"] = out
    fixed = json.dumps(bir).encode()
    nc.to_json_bytes = lambda: fixed
    return nc


def _act_reciprocal(nc, out, in_, tmp):
    """1/x on the Activation engine as exp(-ln(x)) — two ACT passes instead
    of one, but Ln and Exp live together in the natural_log_exp_and_others
    activation-table set (and Copy is in every set), so the softmax's exps,
    these reciprocals, and the PSUM->SBUF cast copies never force a ~1.28us
    ACT_TABLE_LOAD: the hardware Reciprocal function lives in a different
    set and was costing 15 table loads per kernel plus a thrash-induced PE
    stall at every q-tile boundary. Denominators are in [1, ~2.5e3], well
    inside both splines' accurate range."""
    nc.scalar.activation(tmp, in_, mybir.ActivationFunctionType.Ln)
    nc.scalar.activation(out, tmp, mybir.ActivationFunctionType.Exp, scale=-1.0)


def build_core_kernel(s=S, d=D, hq=HQ):
    """Bass module for one core: hq query heads + 1 kv head."""
    nst = s // ST  # s-tiles of 512
    ndk = d // 128  # contraction chunks
    nh = hq + 2  # q heads + k + v
    nnt = d // ST  # output n-tiles

    nqk = hq + 1  # q heads + k (fp8 path)

    nq = 4  # quarters per s-tile
    nquar = nst * nq  # 16 global quarter indices
    ndkq = ndk // nq  # bf16 contraction chunks per quarter (V)
    npair = ndk // 2  # 256-row contraction pair-chunks (DoubleRow)
    npq = npair // nq  # fp8 pair-chunks per quarter (QK)

    nc = bass.Bass()
    # host-pre-swizzled: each [k] / [g] slice is per-partition contiguous
    xT_d = nc.dram_tensor("xT", [nquar * 128, ndkq * ST], BF16, kind="ExternalInput")
    xT8_d = nc.dram_tensor("xT8", [nquar * 128, npq * 2 * ST], FP8, kind="ExternalInput")
    wqk8_d = nc.dram_tensor("wqk8", [(npair // 4) * 128, 4 * 2 * nqk * 128], FP8, kind="ExternalInput")
    wvT_d = nc.dram_tensor("wvT", [(ndk // 8) * 128, 8 * 128], BF16, kind="ExternalInput")
    woT_d = nc.dram_tensor("woT", [hq * 128, d], BF16, kind="ExternalInput")
    cosF_d = nc.dram_tensor("cosF", [128, s], BF16, kind="ExternalInput")
    sinSg_d = nc.dram_tensor("sinSg", [128, s], BF16, kind="ExternalInput")
    maskT_d = nc.dram_tensor("maskT", [128, 128], BF16, kind="ExternalInput")
    ident_d = nc.dram_tensor("ident", [128, 128], BF16, kind="ExternalInput")
    onesr_d = nc.dram_tensor("onesr", [128, 128], BF16, kind="ExternalInput")
    outp_d = nc.dram_tensor("outp", [s, d], BF16, kind="ExternalOutput")

    x8r = xT8_d.rearrange("(k p) (i ko n) -> k p i ko n", p=128, i=npq, ko=2)
    xtr = xT_d.rearrange("(k p) (dk n) -> k p dk n", p=128, dk=ndkq)
    wqk8_r = wqk8_d.rearrange("(g p) (j ko m) -> g p j ko m", p=128, j=4, ko=2)
    wv_r = wvT_d.rearrange("(g p) (c n) -> g p c n", p=128, c=8)

    with tile.TileContext(nc) as tc:
        with (
            tc.tile_pool(name="const", bufs=1) as cp,
            tc.tile_pool(name="qkvsb", bufs=1) as qp,
            tc.tile_pool(name="xp8", bufs=5) as xp8,
            tc.tile_pool(name="xpb", bufs=3) as xpb,
            tc.tile_pool(name="rp", bufs=2) as rp,
            tc.tile_pool(name="vp", bufs=2) as vp,
            tc.tile_pool(name="ep", bufs=15) as ep,
            tc.tile_pool(name="dp", bufs=4) as dpool,
            tc.tile_pool(name="tp", bufs=5) as tpool,
            tc.tile_pool(name="sp", bufs=4) as sp,
            tc.tile_pool(name="op", bufs=2) as op,
            tc.tile_pool(name="at", bufs=8) as atp,
        ):
            # ---- resident constants ----
            # per-chunk weight tiles so the first matmul starts after the
            # first small DMA, not after the whole 10MB weight load
            w8g = [
                cp.tile([128, 4, 2, nqk * 128], FP8, tag=f"w8{g}", name=f"w8{g}")
                for g in range(npair // 4)
            ]
            w8 = [w8g[j // 4][:, j % 4, :, :] for j in range(npair)]
            nwg = ndk // 8  # V weight groups of 8 contraction chunks
            wvsb4 = [
                cp.tile([128, 8, 128], BF16, tag=f"wv{g}", name=f"wv{g}")
                for g in range(nwg)
            ]
            # interleave QK and V weight groups on the async SWDGE (gpsimd)
            # queue: sync-queue (HWDGE) dispatches serialize for the whole
            # transfer, so weights there would starve the x-tile loads
            # w8 group 0 is split so the first matmul's pair-chunk lands
            # after ~150KB instead of ~650KB; its first slice rides the
            # sync queue ahead of x8 — the SWDGE path's first-byte latency
            # is ~2us worse and this pair gates the very first matmul
            nc.sync.dma_start(w8g[0][:, 0:1], wqk8_r[0][:, 0:1])
            nc.gpsimd.dma_start(w8g[0][:, 1:4], wqk8_r[0][:, 1:4])
            nc.gpsimd.dma_start(wvsb4[0], wv_r[0])
            for g in range(1, npair // 4):
                nc.gpsimd.dma_start(w8g[g], wqk8_r[g])
                nc.gpsimd.dma_start(wvsb4[g], wv_r[g])
            wvsb = [wvsb4[dk // 8][:, dk % 8, :] for dk in range(ndk)]
            # everything below is loaded on the SYNC queue, interleaved by
            # hand behind the x tiles it must not starve
            cossb = cp.tile([128, s], BF16)
            sinsb = cp.tile([128, s], BF16)
            masksb = cp.tile([128, 128], BF16)
            identsb = cp.tile([128, 128], BF16)
            onesrsb = cp.tile([128, 128], BF16)
            # wo weight loads are spread across the s-tile loop below: they
            # are only needed in phase B and would otherwise crowd the DMA
            # fabric while the first x tiles load
            wosb = [
                cp.tile([128, d], BF16, tag=f"wo{mh}", name=f"wo{mh}")
                for mh in range(hq)
            ]

            # ---- persistent activations (bf16) ----
            qt_sb = [
                qp.tile([128, s], BF16, tag=f"QT{h}", name=f"QT{h}")
                for h in range(hq)
            ]
            kt_sb = qp.tile([128, s], BF16, tag="KT")
            v_sb = qp.tile([128, s], BF16, tag="V")  # [s%128 part, (s//128)*HD]

            # ---- x tile loads: issued several quarters ahead of compute ----
            x8_tiles = {}
            xt_tiles = {}
            cur = {"x8": 0, "xt": 0}

            def issue_x8():
                k = cur["x8"]
                cur["x8"] += 1
                t = xp8.tile([128, npq, 2, ST], FP8, tag="x8", name=f"x8_{k}")
                if k == 0:
                    # split so the first matmul's x lands after 128KB
                    nc.sync.dma_start(t[:, 0:1], x8r[k][:, 0:1])
                    nc.sync.dma_start(t[:, 1:npq], x8r[k][:, 1:npq])
                else:
                    nc.sync.dma_start(t, x8r[k])
                x8_tiles[k] = t

            def issue_xt():
                k = cur["xt"]
                cur["xt"] += 1
                t = xpb.tile([128, ndkq, ST], BF16, tag="xT", name=f"xt_{k}")
                nc.sync.dma_start(t, xtr[k])
                xt_tiles[k] = t

            def pump(x8_upto, xt_upto):
                # issue in need order (smallest quarter index first, x8
                # before xt at ties): a ring-slot wait on a deep-lookahead
                # x8 dispatch then only delays even-less-urgent dispatches
                x8_upto = min(x8_upto, nquar - 1)
                xt_upto = min(xt_upto, nquar - 1)
                while cur["x8"] <= x8_upto or cur["xt"] <= xt_upto:
                    if cur["xt"] <= xt_upto and cur["xt"] < cur["x8"]:
                        issue_xt()
                    elif cur["x8"] <= x8_upto:
                        issue_x8()
                    else:
                        issue_xt()

            # hand-tuned head of the sync queue: fp8 x first (QK path),
            # small consts woven in where first needed
            issue_x8()  # k=0 (split)
            issue_x8()  # k=1
            issue_xt()  # k=0
            nc.sync.dma_start(identsb, ident_d[:])
            issue_x8()  # k=2
            issue_xt()  # k=1
            nc.sync.dma_start(cossb, cosF_d[:])
            issue_x8()  # k=3
            issue_xt()  # k=2
            nc.sync.dma_start(sinsb, sinSg_d[:])
            nc.scalar.dma_start(masksb, maskT_d[:])
            nc.scalar.dma_start(onesrsb, onesr_d[:])

            # ================= phase A: QKV projection + RoPE =================
            # e_pre holds exp'd score tiles for q-tile 0, computed during
            # phase A (its K/V/Q deps are all s-tile 0) so phase B can start
            # with attnV immediately
            e_pre = {}
            recips0 = {}
            with (
                tc.tile_pool(name="psA", bufs=6, space="PSUM") as psA,
                tc.tile_pool(name="psE", bufs=1, space="PSUM") as psE,
                tc.tile_pool(name="psT", bufs=1, space="PSUM") as psT,
            ):
                # PE clock warmup: the HAM clock gate defaults to 1.2 GHz and
                # only ramps to 2.4 after ~3.4us of sustained matmul activity.
                # Burn the dead time while the first x DMA is in flight on
                # throwaway matmuls so the first real matmuls run warm. The
                # scratch PSUM tile comes from psE (not psA) so the acc ring
                # stays aligned with phase B's bank-alias assumptions.
                warm = cp.tile([128, ST], BF16, tag="warm")
                nc.vector.memset(warm, 1.0)
                warm_ps = psE.tile([128, ST], F32, tag="sce", name="warmps")
                for i in range(18):
                    nc.tensor.matmul(
                        warm_ps, warm[:, 0:128], warm, start=True, stop=True
                    )

                den0 = {}

                def prefetch_attn0_pair(c, h):
                    # scores + exp + mask + den accumulation for q-tile 0,
                    # chunk c (all diagonal), one head. psE is a 1-deep ring
                    # so the next pair's matmul waits on this exp — callers
                    # space the pairs out between QKV head-groups so the
                    # in-order PE queue never stalls on that wait.
                    off = 128 * c
                    w = ST - off
                    sc_ps = psE.tile([128, ST], F32, tag="sce", name=f"sce{c}_{h}")
                    nc.tensor.matmul(
                        sc_ps[:, 0:w],
                        kt_sb[:, c * 128 : (c + 1) * 128],
                        qt_sb[h][:, off:ST],
                        start=True,
                        stop=True,
                    )
                    e_t = ep.tile([128, ST], BF16, tag="E", name=f"e0_{c}_{h}")
                    nc.scalar.activation(
                        e_t[:, 0:w],
                        sc_ps[:, 0:w],
                        mybir.ActivationFunctionType.Exp,
                        scale=1.0 / HD,
                    )
                    nc.vector.tensor_mul(e_t[:, 0:128], e_t[:, 0:128], masksb)
                    e_pre[(c, h)] = e_t
                    if c == 0:
                        den0[h] = dpool.tile(
                            [128, ST], BF16, tag="den", name=f"den0_{h}"
                        )
                        nc.vector.tensor_copy(den0[h], e_t)
                    else:
                        nc.vector.tensor_add(
                            den0[h][:, off:ST], den0[h][:, off:ST], e_t[:, 0:w]
                        )

                pre_queue = [(c, h) for c in range(4) for h in range(hq)]

                def rope_head(acc, h, ssl, swap_dve=False):
                    # RoPE for one head; write bf16. The half-swap copies run
                    # on the scalar engine (partition-shifted copies are legal
                    # there) to cut the DVE chain to 3 ops per head — except
                    # the first head of each s-tile, whose swap goes to DVE:
                    # its acc bank gates the NEXT s-tile's first matmuls and
                    # the scalar engine is jammed with transpose copies then.
                    dst = qt_sb[h] if h < hq else kt_sb
                    t1 = rp.tile([128, ST], BF16, tag="t1")
                    nc.vector.tensor_mul(t1, acc[h], cossb[:, ssl])
                    tsw = rp.tile([128, ST], BF16, tag="tsw")
                    eng = nc.vector if swap_dve else nc.scalar
                    if swap_dve:
                        nc.vector.tensor_copy(tsw[0:64, :], acc[h][64:128, :])
                        nc.vector.tensor_copy(tsw[64:128, :], acc[h][0:64, :])
                    else:
                        nc.scalar.copy(tsw[0:64, :], acc[h][64:128, :])
                        nc.scalar.copy(tsw[64:128, :], acc[h][0:64, :])
                    nc.vector.tensor_mul(tsw, tsw, sinsb[:, ssl])
                    nc.vector.tensor_add(dst[:, ssl], t1, tsw)

                for st in range(nst):
                    ssl = slice(st * ST, (st + 1) * ST)
                    acc = [
                        psA.tile([128, ST], F32, tag="acc", name=f"acc{h}")
                        for h in range(nh)
                    ]

                    def qk_head_quar(h, quar, x8a):
                        for i in range(npq):
                            nc.tensor.matmul(
                                acc[h],
                                w8[quar * npq + i][:, :, h * 128 : (h + 1) * 128],
                                x8a[:, i, :, :],
                                start=(quar == 0 and i == 0),
                                stop=(quar == nq - 1 and i == npq - 1),
                                perf_mode=mybir.MatmulPerfMode.DoubleRow,
                            )

                    def v_mms(quar, xta):
                        for dk in range(ndkq):
                            nc.tensor.matmul(
                                acc[nh - 1],
                                wvsb[quar * ndkq + dk],
                                xta[:, dk, :],
                                start=(quar == 0 and dk == 0),
                                stop=(quar == nq - 1 and dk == ndkq - 1),
                            )

                    def transpose_v(st):
                        # V: transpose [HD, s-tile] -> [s-chunk, HD] blocks.
                        # All copies on the scalar engine so the transpose
                        # chain (and everything behind it on the in-order PE
                        # queue) doesn't stall on the DVE RoPE backlog.
                        for j in range(ST // 128):
                            vtmp = vp.tile([128, 128], BF16, tag="vtmp")
                            nc.scalar.copy(
                                vtmp, acc[hq + 1][:, j * 128 : (j + 1) * 128]
                            )
                            tp_ps = psT.tile([128, 128], BF16, tag="tp")
                            nc.tensor.transpose(tp_ps, vtmp, identsb)
                            sc = st * (ST // 128) + j
                            nc.scalar.copy(
                                v_sb[:, sc * 128 : (sc + 1) * 128], tp_ps
                            )

                    if st < nst - 1:
                        for quar in range(nq):
                            k = st * nq + quar
                            pump(k + 4, k + 1)
                            # QK for this quarter, with the q-tile 0 attn
                            # prefetch pairs spaced between head-groups of
                            # s-tile 2; V runs one quarter STAGGERED so its
                            # xt tile and wv weights get an extra quarter of
                            # DMA slack (matters most in s-tile 0)
                            for h in range(nqk):
                                qk_head_quar(h, quar, x8_tiles[k])
                                if st == 2 and pre_queue:
                                    prefetch_attn0_pair(*pre_queue.pop(0))
                            v_mms(quar, xt_tiles[k])
                        # wo weights behind this s-tile's x loads
                        nc.sync.dma_start(
                            wosb[st], woT_d[st * 128 : (st + 1) * 128, :]
                        )
                        transpose_v(st)
                        for h in range(hq + 1):
                            rope_head(acc, h, ssl)
                    else:
                        # ---- last s-tile: head-major so each head's RoPE
                        # (and the PSUM bank phase B's attnV q-tile 0 reuses)
                        # completes while later heads still stream ----
                        pump(nquar - 1, nquar - 1)
                        nc.sync.dma_start(
                            wosb[st], woT_d[st * 128 : (st + 1) * 128, :]
                        )
                        base = st * nq
                        for h in range(hq):
                            for quar in range(nq):
                                qk_head_quar(h, quar, x8_tiles[base + quar])
                            rope_head(acc, h, ssl)
                        # q-tile 0 denominator broadcast + reciprocal, woven
                        # between the K quarter-groups so the PE never waits
                        # on the 1-deep psE ring (each bc's reciprocal runs
                        # on ACT while the next K quarter streams)
                        for quar in range(nq):
                            qk_head_quar(hq, quar, x8_tiles[base + quar])
                            bc_ps = psE.tile(
                                [128, ST], F32, tag="sce", name=f"bc0_{quar}"
                            )
                            nc.tensor.matmul(
                                bc_ps, onesrsb, den0[quar], start=True, stop=True
                            )
                            recip = sp.tile(
                                [128, ST], F32, tag="recip", name=f"recip0_{quar}"
                            )
                            lntmp = sp.tile(
                                [128, ST], F32, tag="lntmp", bufs=1,
                                name=f"lnt0_{quar}",
                            )
                            _act_reciprocal(nc, recip, bc_ps, lntmp)
                            recips0[quar] = recip
                        rope_head(acc, hq, ssl)
                        for quar in range(nq):
                            v_mms(quar, xt_tiles[base + quar])
                        transpose_v(st)

            # ============ phase B: attention + output projection ============
            # pool order matters: psAt's banks alias phase A's Q-head acc
            # banks (freed as each head's RoPE completes in the head-major
            # last s-tile), so attnV for the prefetched q-tile 0 can start
            # before the K/V epilogue finishes
            with (
                tc.tile_pool(name="psAt", bufs=4, space="PSUM") as psAt,
                tc.tile_pool(name="psS", bufs=2, space="PSUM") as psS,
                tc.tile_pool(name="psW", bufs=2, space="PSUM") as psW,
            ):
                cast_ctr = [0]
                osb_cur = [None]

                def emit_wo_task(qt, attn_tiles, j, nt):
                    # one output tile of wo for q-tile qt: 4 matmuls
                    # (contraction over the 4 heads) + cast. Four consecutive
                    # nt tiles share one [128, 2048] osb buffer flushed by a
                    # single DMA; the very last s-chunk flushes every 2 tiles
                    # to shorten the drain tail.
                    sc = qt * (ST // 128) + j
                    last_sc = sc == nst * (ST // 128) - 1
                    gran = 2 if last_sc else 4
                    o_ps = psW.tile(
                        [128, ST], F32, tag="wops", name=f"wo{qt}_{j}_{nt}"
                    )
                    for mh in range(hq):
                        nc.tensor.matmul(
                            o_ps,
                            attn_tiles[mh][:, j * 128 : (j + 1) * 128],
                            wosb[mh][:, nt * ST : (nt + 1) * ST],
                            start=(mh == 0),
                            stop=(mh == hq - 1),
                        )
                    if nt % gran == 0:
                        osb_cur[0] = op.tile(
                            [128, 4 * ST], BF16, tag="osb",
                            name=f"osb{qt}_{j}_{nt}",
                        )
                    osb = osb_cur[0]
                    # alternate the PSUM->SBUF cast between ACT and DVE
                    cast_ctr[0] += 1
                    if cast_ctr[0] % 2 == 0:
                        nc.scalar.copy(osb[:, (nt % gran) * ST : (nt % gran + 1) * ST], o_ps)
                    else:
                        nc.vector.tensor_copy(
                            osb[:, (nt % gran) * ST : (nt % gran + 1) * ST], o_ps
                        )
                    if nt % gran == gran - 1:
                        nc.sync.dma_start(
                            outp_d[
                                sc * 128 : (sc + 1) * 128,
                                (nt - gran + 1) * ST : (nt + 1) * ST,
                            ],
                            osb[:, 0 : gran * ST],
                        )

                pending = []  # wo tasks of the previous q-tile
                boundary = []  # held-back tasks two q-tiles old: already
                # normalized, so they can fill the PE during the next
                # q-tile's first chunks while everything else waits on ACT
                deferred_fin = []  # prev q-tile's recip+normalize closures
                for qt in range(nst):
                    nk = (qt + 1) * (ST // 128)  # causal: k chunks this q-tile
                    with nc.named_scope(f"attn{qt}"):
                        at_tiles = {
                            h: psAt.tile([128, ST], F32, tag="at", name=f"at{qt}_{h}")
                            for h in range(hq)
                        }
                        # den accumulation state, all off the PE:
                        # binary-counter tree over full chunks (2x-rate DVE
                        # bf16 adds; GpSimd takes one head), merged into a
                        # single running total when the full chunks end, then
                        # diagonal chunks added in place at their offset as
                        # they arrive — so the q-tile boundary only pays one
                        # add + one broadcast matmul + one reciprocal.
                        tree = {h: {} for h in range(hq)}
                        tot = {}
                        attn_tiles = {}
                        fins = []
                        if qt == 0:
                            # q-tile 0 was fully prefetched in phase A; emit
                            # its attnV head-major so head h's chain issues
                            # as soon as RoPE(Qh) frees its aliased PSUM
                            # bank, instead of gating on all four heads
                            for h in range(hq):
                                for c in range(nk):
                                    o2 = 128 * c
                                    nc.tensor.matmul(
                                        at_tiles[h][:, o2:ST],
                                        v_sb[:, c * 128 : (c + 1) * 128],
                                        e_pre[(c, h)][:, 0 : ST - o2],
                                        start=(c == 0),
                                        stop=(c == nk - 1),
                                    )
                        for c in range(0 if qt == 0 else nk):
                            # diagonal chunks: only columns >= 128*r valid
                            r = c - (nk - 4)
                            off = 128 * r if r > 0 else 0
                            w = ST - off
                            last = c == nk - 1
                            # wo matmuls of the previous q-tile fill the PE
                            # while this chunk's exps run on the scalar
                            # engine; emit them BETWEEN the scores matmuls
                            # (the psS ring is 2 deep, so scores h=2 waits on
                            # exp h=0 — fillers keep the in-order PE queue
                            # fed). Gated until the previous q-tile's
                            # deferred normalize closures have all run: the
                            # wo matmuls read the in-place-normalized tiles.
                            fillers = []
                            if boundary and c < 2:
                                take = min(4, len(boundary))
                                fillers += [boundary.pop(0) for _ in range(take)]
                            if pending and not deferred_fin:
                                hold = 8 if qt < nst - 1 else 0
                                n_emit = -(-max(0, len(pending) - hold) // (nk - c))
                                fillers += [pending.pop(0) for _ in range(n_emit)]
                            nf2 = len(fillers) // 2
                            first_emitted = False
                            e_ts = {}
                            for h in range(hq):
                                sc_ps = psS.tile(
                                    [128, ST], F32, tag="sc",
                                    name=f"sc{qt}_{c}_{h}",
                                )
                                nc.tensor.matmul(
                                    sc_ps[:, 0:w],
                                    kt_sb[:, c * 128 : (c + 1) * 128],
                                    qt_sb[h][:, qt * ST + off : (qt + 1) * ST],
                                    start=True,
                                    stop=True,
                                )
                                e_t = ep.tile(
                                    [128, ST], BF16, tag="E",
                                    name=f"e{qt}_{c}_{h}",
                                )
                                nc.scalar.activation(
                                    e_t[:, 0:w],
                                    sc_ps[:, 0:w],
                                    mybir.ActivationFunctionType.Exp,
                                    scale=1.0 / HD,
                                )
                                if r >= 0:
                                    nc.vector.tensor_mul(
                                        e_t[:, 0:128], e_t[:, 0:128], masksb
                                    )
                                e_ts[h] = e_t
                                if h == 1:
                                    first_emitted = True
                                    for t in fillers[:nf2]:
                                        emit_wo_task(*t)
                            for t in (fillers[nf2:] if first_emitted else fillers):
                                emit_wo_task(*t)
                            for h in range(hq):
                                nc.tensor.matmul(
                                    at_tiles[h][:, off:ST],
                                    v_sb[:, c * 128 : (c + 1) * 128],
                                    e_ts[h][:, 0:w],
                                    start=(c == 0),
                                    stop=(c == nk - 1),
                                )
                            # previous q-tile's deferred reciprocal+normalize
                            # closures: two after each of the first chunks
                            if deferred_fin and c < 2:
                                for _ in range(min(2, len(deferred_fin))):
                                    deferred_fin.pop(0)()
                            # ---- denominator accumulation (off the PE) ----
                            for h in range(hq):
                                eng = nc.vector if h < 3 else nc.gpsimd
                                if off == 0:
                                    # full chunk: binary-counter tree insert
                                    carry = e_ts[h]
                                    lvl = 0
                                    while lvl < 3 and tree[h].get(lvl) is not None:
                                        t_new = tpool.tile(
                                            [128, ST], BF16, tag=f"tr{h}",
                                            name=f"tr{qt}_{c}_{h}_{lvl}",
                                        )
                                        eng.tensor_add(t_new, tree[h][lvl], carry)
                                        tree[h][lvl] = None
                                        carry = t_new
                                        lvl += 1
                                    tree[h][lvl] = carry
                                    if c == nk - 4:
                                        # full chunks end here: merge the
                                        # tree into one running total
                                        lvls = [
                                            l for l in sorted(tree[h])
                                            if tree[h][l] is not None
                                        ]
                                        t_tot = tree[h][lvls[0]]
                                        for l in lvls[1:]:
                                            t_new = tpool.tile(
                                                [128, ST], BF16, tag=f"tr{h}",
                                                name=f"tm{qt}_{c}_{h}_{l}",
                                            )
                                            eng.tensor_add(
                                                t_new, t_tot, tree[h][l]
                                            )
                                            t_tot = t_new
                                        tot[h] = t_tot
                                        tree[h] = {}
                                else:
                                    # diagonal chunk: add in place at offset
                                    eng.tensor_add(
                                        tot[h][:, off:ST],
                                        tot[h][:, off:ST],
                                        e_ts[h][:, 0:w],
                                    )
                                if last:
                                    # free the attnV PSUM bank NOW with a
                                    # plain bf16 copy (no reciprocal dep),
                                    # then defer reduce+reciprocal+in-place
                                    # normalize into the NEXT q-tile's first
                                    # chunks, where ACT interleaves them
                                    # with that q-tile's exps instead of
                                    # serializing at the boundary
                                    atn = atp.tile(
                                        [128, ST], BF16, tag="attnT",
                                        name=f"atn{qt}_{h}",
                                    )
                                    nc.vector.tensor_copy(atn, at_tiles[h])
                                    attn_tiles[h] = atn

                                    def make_fin(tot_h, atn_h, qi, hh):
                                        def fin():
                                            bc_ps = psW.tile(
                                                [128, ST], F32, tag="wops",
                                                name=f"bc{qi}_{hh}",
                                            )
                                            nc.tensor.matmul(
                                                bc_ps, onesrsb, tot_h,
                                                start=True, stop=True,
                                            )
                                            recip = sp.tile(
                                                [128, ST], F32, tag="recip",
                                                name=f"recip{qi}_{hh}",
                                            )
                                            lntmp = sp.tile(
                                                [128, ST], F32, tag="lntmp",
                                                bufs=1, name=f"lnt{qi}_{hh}",
                                            )
                                            _act_reciprocal(
                                                nc, recip, bc_ps, lntmp
                                            )
                                            nc.vector.tensor_mul(
                                                atn_h, atn_h, recip
                                            )
                                        return fin

                                    fins.append(make_fin(tot[h], atn, qt, h))
                        boundary += pending  # <=8 kept-back tasks
                        pending = []
                        if qt == 0:
                            # q-tile 0's reciprocals were computed in phase
                            # A: copy + normalize in place right away
                            for h in range(hq):
                                atn = atp.tile(
                                    [128, ST], BF16, tag="attnT",
                                    name=f"atn0_{h}",
                                )
                                nc.vector.tensor_copy(atn, at_tiles[h])
                                nc.vector.tensor_mul(atn, atn, recips0[h])
                                attn_tiles[h] = atn
                        if qt == nst - 1:
                            # no next q-tile to defer into
                            for fin in fins:
                                fin()
                            fins = []
                        deferred_fin = fins
                    pending = [
                        (qt, attn_tiles, j, nt)
                        for j in range(ST // 128)
                        for nt in range(nnt)
                    ]
                for t in boundary + pending:
                    emit_wo_task(*t)
    return _legalize_single_wait(nc)


def host_prep(x, wq, wk, wv, wo, s=S, d=D, hq=HQ, ncores=NCORES):
    """Shared tensors + per-core weight shards, all host-side numpy.

    Every big tensor is pre-swizzled into exactly the SBUF tile layout the
    kernel loads, so each dma_start moves a per-partition-contiguous block:
      xT8  [16*128, 4096]: row k*128+p, col i*1024+ko*512+n
                           <- x[d=quar*1024+i*256+ko*128+p, s=st*512+n]
      xT   [16*128, 4096]: col dk*512+n <- x[d=quar*1024+dk*128+p, ...]
      wqk8 [4*128, 5120]:  row g*128+p, col j*1280+ko*640+m
                           <- w[d=g*1024+j*256+ko*128+p, m]
      wvT  [4*128, 1024]:  col c*128+n <- w[d=g*1024+c*128+p, n]
    """
    scale = attn_scale(s, HD, MULT)
    xTf = np.ascontiguousarray(x.reshape(s, d).T)
    xT = xTf.astype(NPBF16)
    xT8 = xTf.astype(NPFP8)

    # (quar, i, ko, p, st, n) -> (st, quar, p, i, ko, n)
    x8L = np.ascontiguousarray(
        xT8.reshape(4, 4, 2, 128, 4, 512).transpose(4, 0, 3, 1, 2, 5)
    ).reshape(16 * 128, 4096)
    # (quar, dk, p, st, n) -> (st, quar, p, dk, n)
    xTL = np.ascontiguousarray(
        xT.reshape(4, 8, 128, 4, 512).transpose(3, 0, 2, 1, 4)
    ).reshape(16 * 128, 4096)

    freq = ROPE_BASE ** (-(np.arange(0, HD, 2, dtype=np.float64) / HD))
    pos = np.arange(s, dtype=np.float64)
    angle = pos[:, None] * freq[None, :]  # [s, 64]
    cos = np.cos(angle).astype(NPBF16).T  # [64, s]
    sin = np.sin(angle).astype(NPBF16).T
    cosF = np.ascontiguousarray(np.concatenate([cos, cos], axis=0))
    sinSg = np.ascontiguousarray(np.concatenate([-sin, sin], axis=0))

    # triangular causal mask for diagonal chunks: keep iff p <= f
    p = np.arange(128)[:, None]
    f = np.arange(128)[None, :]
    maskT = (p <= f).astype(NPBF16)  # [128, 128]

    ident = np.eye(128, dtype=NPBF16)
    onesr = np.ones((128, 128), dtype=NPBF16)

    shared = dict(
        xT=xTL, xT8=x8L, cosF=cosF, sinSg=sinSg, maskT=maskT, ident=ident,
        onesr=onesr,
    )

    in_maps = []
    for c in range(ncores):
        wq_c = wq[c * hq * 128 : (c + 1) * hq * 128, :]  # [hq*128, d]
        wk_c = wk[c * 128 : (c + 1) * 128, :]
        wv_c = wv[c * 128 : (c + 1) * 128, :] * scale
        wqk8 = np.ascontiguousarray(
            np.concatenate([wq_c.T, wk_c.T], axis=1)
        ).astype(NPFP8)  # [d, (hq+1)*128]
        # (g, j, ko, p, m) -> (g, p, j, ko, m)
        w8L = np.ascontiguousarray(
            wqk8.reshape(4, 4, 2, 128, 640).transpose(0, 3, 1, 2, 4)
        ).reshape(4 * 128, 5120)
        wvT = np.ascontiguousarray(wv_c.T).astype(NPBF16)  # [d, 128]
        # (g, c, p, n) -> (g, p, c, n)
        wvL = np.ascontiguousarray(
            wvT.reshape(4, 8, 128, 128).transpose(0, 2, 1, 3)
        ).reshape(4 * 128, 1024)
        wo_c = wo[:, c * hq * 128 : (c + 1) * hq * 128]  # [d, hq*128]
        woT = np.ascontiguousarray(wo_c.T).astype(NPBF16)  # [hq*128, d]
        in_maps.append(dict(shared, wqk8=w8L, wvT=wvL, woT=woT))
    return in_maps


_NC_CACHE = {}


def kernel(x, freqs_cis, wq, wk, wv, wo):
    del freqs_cis  # forward pass recomputes rope tables (matches reference)
    x = np.asarray(x, dtype=np.float32)
    key = (S, D, HQ)
    if key not in _NC_CACHE:
        _NC_CACHE[key] = build_core_kernel(S, D, HQ)
    nc = _NC_CACHE[key]
    in_maps = host_prep(
        x, np.asarray(wq, np.float32), np.asarray(wk, np.float32),
        np.asarray(wv, np.float32), np.asarray(wo, np.float32),
    )
    res = run_bass_kernel_spmd(nc, in_maps, core_ids=list(range(NCORES)))
    out = np.zeros((S, D), dtype=np.float32)
    for r in res.results:
        out += np.asarray(r["outp"], dtype=np.float32)
    return out.reshape(B, S, D)


if __name__ == "__main__":
    rng = np.random.default_rng(0)
    x = rng.standard_normal((B, S, D)).astype(np.float32)
    wq = (rng.standard_normal((H * HD, D)) * D**-0.5).astype(np.float32)
    wk = (rng.standard_normal((KVH * HD, D)) * D**-0.5).astype(np.float32)
    wv = (rng.standard_normal((KVH * HD, D)) * D**-0.5).astype(np.float32)
    wo = (rng.standard_normal((D, H * HD)) * (H * HD) ** -0.5).astype(np.float32)
    fc = rng.standard_normal((S, HD // 2)).astype(np.float32)
    out = kernel(x, fc, wq, wk, wv, wo)
    print(out.shape, out.dtype, np.abs(out).max())


# revision 54
# speedup vs baseline: 1.0181x; 1.0002x over previous
"""GQA attention layer (B=1, S=2048, D=4096, H=32, KVH=8, HD=128) on 8 TRN2
NeuronCores, tensor-parallel over heads.

Each core computes 4 query heads + their shared kv head end-to-end:
QKV projection -> RoPE -> causal attention (no-max-sub softmax, scores are
tiny) -> its slice of the wo projection. The 8 partial [S, D] outputs are
summed on the host (the "all-reduce after wo" of the sharding hint).

Device layouts (everything bf16 into the PE, fp32 PSUM accumulation):
  QT/KT  [HD=128(part), S]    from  lhsT=w[d,:], rhs=xT[d, s-tile]
  V      [S(part), HD]        via PE-transpose of VT
  scoresT[k(part), q]         lhsT=KT chunk, rhs=QT tile
  E = exp(scoresT/128) bf16; causal diagonal via 0/1 mask multiply
  attnT  [HD(part), q]        lhsT=V chunk, rhs=E  (accumulated over k)
  den    [128, q] bf16 SBUF   accumulated on DVE/GpSimd (off PE)
  den reduce+broadcast        one matmul lhsT=ones[128,128], rhs=den
  attnT_norm = attnT * recip  (DVE mul, bf16 out)
  out    [s(part), n]         lhsT=attnT_norm chunk, rhs=woT

All DMA-heavy tensors are pre-swizzled on the host into exactly the SBUF
tile layout (per-partition contiguous), so every dma_start is a plain
[128, contiguous] block: cheap to dispatch and line-rate to transfer.
x tile loads run several quarters ahead of compute on the sync queue.

The last s-tile runs its QK matmuls head-major so each head's RoPE (and the
PSUM bank it frees for phase B's prefetched q-tile-0 attnV) completes while
later heads' matmuls still stream; q-tile 0's softmax denominators +
reciprocals are also emitted inside the phase-A tail, hiding the ACT
reciprocal table switch.

wo matmuls for q-tile t-1 are interleaved between the scores and attnV
matmuls of q-tile t so the PE fills the exp-wait gaps (the scalar engine's
exps per chunk exceed the attention matmul time per chunk).
"""

import json
import math

import ml_dtypes
import numpy as np

import concourse.bass as bass
import concourse.tile as tile
from concourse import mybir
from concourse.bass_utils import run_bass_kernel_spmd

BF16 = mybir.dt.bfloat16
F32 = mybir.dt.float32
FP8 = mybir.dt.float8e4
NPBF16 = ml_dtypes.bfloat16
NPFP8 = ml_dtypes.float8_e4m3

# Full problem constants
B, S, D = 1, 2048, 4096
H, KVH = 32, 8
HD = 128
NCORES = 8
HQ = H // NCORES  # query heads per core
MULT = 1.0
ROPE_BASE = 10000.0
ST = 512  # s-tile (PSUM bank width in fp32)


def attn_scale(seq_len=S, d_head=HD, mult=MULT):
    alpha = 1.0 / (1.0 + 4.0 * d_head / mult**2)
    lower = (math.log(seq_len) / seq_len) ** 0.5
    interp = math.exp((1.0 - alpha) * math.log(lower))
    return 1.0 / interp


def _legalize_single_wait(nc):
    """The walrus build in this container accepts only ONE sync wait per
    instruction ("Too many sync wait commands" in setupSyncWait). Split
    extra waits into preceding single-wait Drains (lowered to CTRL NOPs)
    on the same engine — same in-order stall semantics."""
    bir = json.loads(nc.to_json_bytes())
    ctr = 0
    for fn in bir["functions"]:
        for blk in fn["blocks"]:
            out = []
            for inst in blk["instructions"]:
                si = inst.get("sync_info")
                waits = (si or {}).get("on_wait") or []
                if len(waits) > 1:
                    for w in waits[:-1]:
                        ctr += 1
                        out.append(
                            {
                                "debug": inst.get("debug", 0),
                                "engine": inst["engine"],
                                "ins": [],
                                "name": f"{inst['name']}-mw{ctr}",
                                "opcode": "Drain",
                                "outs": [],
                                "sync_info": {"on_update": [], "on_wait": [w]},
                            }
                        )
                    si["on_wait"] = [waits[-1]]
                out.append(inst)
            blk["instructions"] = out
    fixed = json.dumps(bir).encode()
    nc.to_json_bytes = lambda: fixed
    return nc


def _act_reciprocal(nc, out, in_, tmp):
    """1/x on the Activation engine as exp(-ln(x)) — two ACT passes instead
    of one, but Ln and Exp live together in the natural_log_exp_and_others
    activation-table set (and Copy is in every set), so the softmax's exps,
    these reciprocals, and the PSUM->SBUF cast copies never force a ~1.28us
    ACT_TABLE_LOAD: the hardware Reciprocal function lives in a different
    set and was costing 15 table loads per kernel plus a thrash-induced PE
    stall at every q-tile boundary. Denominators are in [1, ~2.5e3], well
    inside both splines' accurate range."""
    nc.scalar.activation(tmp, in_, mybir.ActivationFunctionType.Ln)
    nc.scalar.activation(out, tmp, mybir.ActivationFunctionType.Exp, scale=-1.0)


def build_core_kernel(s=S, d=D, hq=HQ):
    """Bass module for one core: hq query heads + 1 kv head."""
    nst = s // ST  # s-tiles of 512
    ndk = d // 128  # contraction chunks
    nh = hq + 2  # q heads + k + v
    nnt = d // ST  # output n-tiles

    nqk = hq + 1  # q heads + k (fp8 path)

    nq = 4  # quarters per s-tile
    nquar = nst * nq  # 16 global quarter indices
    ndkq = ndk // nq  # bf16 contraction chunks per quarter (V)
    npair = ndk // 2  # 256-row contraction pair-chunks (DoubleRow)
    npq = npair // nq  # fp8 pair-chunks per quarter (QK)

    nc = bass.Bass()
    # host-pre-swizzled: each [k] / [g] slice is per-partition contiguous
    xT_d = nc.dram_tensor("xT", [nquar * 128, ndkq * ST], BF16, kind="ExternalInput")
    xT8_d = nc.dram_tensor("xT8", [nquar * 128, npq * 2 * ST], FP8, kind="ExternalInput")
    wqk8_d = nc.dram_tensor("wqk8", [(npair // 4) * 128, 4 * 2 * nqk * 128], FP8, kind="ExternalInput")
    wvT_d = nc.dram_tensor("wvT", [(ndk // 8) * 128, 8 * 128], BF16, kind="ExternalInput")
    woT_d = nc.dram_tensor("woT", [hq * 128, d], BF16, kind="ExternalInput")
    cosF_d = nc.dram_tensor("cosF", [128, s], BF16, kind="ExternalInput")
    sinSg_d = nc.dram_tensor("sinSg", [128, s], BF16, kind="ExternalInput")
    maskT_d = nc.dram_tensor("maskT", [128, 128], BF16, kind="ExternalInput")
    ident_d = nc.dram_tensor("ident", [128, 128], BF16, kind="ExternalInput")
    onesr_d = nc.dram_tensor("onesr", [128, 128], BF16, kind="ExternalInput")
    outp_d = nc.dram_tensor("outp", [s, d], BF16, kind="ExternalOutput")

    x8r = xT8_d.rearrange("(k p) (i ko n) -> k p i ko n", p=128, i=npq, ko=2)
    xtr = xT_d.rearrange("(k p) (dk n) -> k p dk n", p=128, dk=ndkq)
    wqk8_r = wqk8_d.rearrange("(g p) (j ko m) -> g p j ko m", p=128, j=4, ko=2)
    wv_r = wvT_d.rearrange("(g p) (c n) -> g p c n", p=128, c=8)

    with tile.TileContext(nc) as tc:
        with (
            tc.tile_pool(name="const", bufs=1) as cp,
            tc.tile_pool(name="qkvsb", bufs=1) as qp,
            tc.tile_pool(name="xp8", bufs=5) as xp8,
            tc.tile_pool(name="xpb", bufs=3) as xpb,
            tc.tile_pool(name="rp", bufs=2) as rp,
            tc.tile_pool(name="vp", bufs=2) as vp,
            tc.tile_pool(name="ep", bufs=15) as ep,
            tc.tile_pool(name="dp", bufs=4) as dpool,
            tc.tile_pool(name="tp", bufs=5) as tpool,
            tc.tile_pool(name="sp", bufs=4) as sp,
            tc.tile_pool(name="op", bufs=2) as op,
            tc.tile_pool(name="at", bufs=8) as atp,
        ):
            # ---- resident constants ----
            # per-chunk weight tiles so the first matmul starts after the
            # first small DMA, not after the whole 10MB weight load
            w8g = [
                cp.tile([128, 4, 2, nqk * 128], FP8, tag=f"w8{g}", name=f"w8{g}")
                for g in range(npair // 4)
            ]
            w8 = [w8g[j // 4][:, j % 4, :, :] for j in range(npair)]
            nwg = ndk // 8  # V weight groups of 8 contraction chunks
            wvsb4 = [
                cp.tile([128, 8, 128], BF16, tag=f"wv{g}", name=f"wv{g}")
                for g in range(nwg)
            ]
            # interleave QK and V weight groups on the async SWDGE (gpsimd)
            # queue: sync-queue (HWDGE) dispatches serialize for the whole
            # transfer, so weights there would starve the x-tile loads
            # w8 group 0 is split so the first matmul's pair-chunk lands
            # after ~150KB instead of ~650KB; its first slice rides the
            # sync queue ahead of x8 — the SWDGE path's first-byte latency
            # is ~2us worse and this pair gates the very first matmul
            nc.sync.dma_start(w8g[0][:, 0:1], wqk8_r[0][:, 0:1])
            nc.gpsimd.dma_start(w8g[0][:, 1:4], wqk8_r[0][:, 1:4])
            nc.gpsimd.dma_start(wvsb4[0], wv_r[0])
            for g in range(1, npair // 4):
                nc.gpsimd.dma_start(w8g[g], wqk8_r[g])
                nc.gpsimd.dma_start(wvsb4[g], wv_r[g])
            wvsb = [wvsb4[dk // 8][:, dk % 8, :] for dk in range(ndk)]
            # everything below is loaded on the SYNC queue, interleaved by
            # hand behind the x tiles it must not starve
            cossb = cp.tile([128, s], BF16)
            sinsb = cp.tile([128, s], BF16)
            masksb = cp.tile([128, 128], BF16)
            identsb = cp.tile([128, 128], BF16)
            onesrsb = cp.tile([128, 128], BF16)
            # wo weight loads are spread across the s-tile loop below: they
            # are only needed in phase B and would otherwise crowd the DMA
            # fabric while the first x tiles load
            wosb = [
                cp.tile([128, d], BF16, tag=f"wo{mh}", name=f"wo{mh}")
                for mh in range(hq)
            ]

            # ---- persistent activations (bf16) ----
            qt_sb = [
                qp.tile([128, s], BF16, tag=f"QT{h}", name=f"QT{h}")
                for h in range(hq)
            ]
            kt_sb = qp.tile([128, s], BF16, tag="KT")
            v_sb = qp.tile([128, s], BF16, tag="V")  # [s%128 part, (s//128)*HD]

            # ---- x tile loads: issued several quarters ahead of compute ----
            x8_tiles = {}
            xt_tiles = {}
            cur = {"x8": 0, "xt": 0}

            def issue_x8():
                k = cur["x8"]
                cur["x8"] += 1
                t = xp8.tile([128, npq, 2, ST], FP8, tag="x8", name=f"x8_{k}")
                if k == 0:
                    # split so the first matmul's x lands after 128KB
                    nc.sync.dma_start(t[:, 0:1], x8r[k][:, 0:1])
                    nc.sync.dma_start(t[:, 1:npq], x8r[k][:, 1:npq])
                else:
                    nc.sync.dma_start(t, x8r[k])
                x8_tiles[k] = t

            def issue_xt():
                k = cur["xt"]
                cur["xt"] += 1
                t = xpb.tile([128, ndkq, ST], BF16, tag="xT", name=f"xt_{k}")
                nc.sync.dma_start(t, xtr[k])
                xt_tiles[k] = t

            def pump(x8_upto, xt_upto):
                # issue in need order (smallest quarter index first, x8
                # before xt at ties): a ring-slot wait on a deep-lookahead
                # x8 dispatch then only delays even-less-urgent dispatches
                x8_upto = min(x8_upto, nquar - 1)
                xt_upto = min(xt_upto, nquar - 1)
                while cur["x8"] <= x8_upto or cur["xt"] <= xt_upto:
                    if cur["xt"] <= xt_upto and cur["xt"] < cur["x8"]:
                        issue_xt()
                    elif cur["x8"] <= x8_upto:
                        issue_x8()
                    else:
                        issue_xt()

            # hand-tuned head of the sync queue: fp8 x first (QK path),
            # small consts woven in where first needed
            issue_x8()  # k=0 (split)
            issue_x8()  # k=1
            issue_xt()  # k=0
            nc.sync.dma_start(identsb, ident_d[:])
            issue_x8()  # k=2
            issue_xt()  # k=1
            nc.sync.dma_start(cossb, cosF_d[:])
            issue_x8()  # k=3
            issue_xt()  # k=2
            nc.sync.dma_start(sinsb, sinSg_d[:])
            nc.scalar.dma_start(masksb, maskT_d[:])
            nc.scalar.dma_start(onesrsb, onesr_d[:])

            # ================= phase A: QKV projection + RoPE =================
            # e_pre holds exp'd score tiles for q-tile 0, computed during
            # phase A (its K/V/Q deps are all s-tile 0) so phase B can start
            # with attnV immediately
            e_pre = {}
            recips0 = {}
            with (
                tc.tile_pool(name="psA", bufs=6, space="PSUM") as psA,
                tc.tile_pool(name="psE", bufs=1, space="PSUM") as psE,
                tc.tile_pool(name="psT", bufs=1, space="PSUM") as psT,
            ):
                # PE clock warmup: the HAM clock gate defaults to 1.2 GHz and
                # only ramps to 2.4 after ~3.4us of sustained matmul activity.
                # Burn the dead time while the first x DMA is in flight on
                # throwaway matmuls so the first real matmuls run warm. The
                # scratch PSUM tile comes from psE (not psA) so the acc ring
                # stays aligned with phase B's bank-alias assumptions.
                warm = cp.tile([128, ST], BF16, tag="warm")
                nc.vector.memset(warm, 1.0)
                warm_ps = psE.tile([128, ST], F32, tag="sce", name="warmps")
                for i in range(18):
                    nc.tensor.matmul(
                        warm_ps, warm[:, 0:128], warm, start=True, stop=True
                    )

                dummy_ctr = [0]

                def dummy_mms(target, n):
                    # dependency-light garbage matmuls that keep the HAM
                    # activity window busy across known sub-2us PE stalls:
                    # without them every such stall drops the PE clock to
                    # 1.2GHz for the next ~3.4us (measured ~24us/kernel of
                    # half-speed matmuls). Writing a PSUM region that a
                    # later start=True group overwrites is harmless.
                    for _ in range(n):
                        dummy_ctr[0] += 1
                        nc.tensor.matmul(
                            target[:, 0:128],
                            warm[:, 0:128],
                            warm[:, 0:128],
                            start=True,
                            stop=True,
                        )

                den0 = {}

                def prefetch_attn0_pair(c, h):
                    # scores + exp + mask + den accumulation for q-tile 0,
                    # chunk c (all diagonal), one head. psE is a 1-deep ring
                    # so the next pair's matmul waits on this exp — callers
                    # space the pairs out between QKV head-groups so the
                    # in-order PE queue never stalls on that wait.
                    off = 128 * c
                    w = ST - off
                    sc_ps = psE.tile([128, ST], F32, tag="sce", name=f"sce{c}_{h}")
                    nc.tensor.matmul(
                        sc_ps[:, 0:w],
                        kt_sb[:, c * 128 : (c + 1) * 128],
                        qt_sb[h][:, off:ST],
                        start=True,
                        stop=True,
                    )
                    e_t = ep.tile([128, ST], BF16, tag="E", name=f"e0_{c}_{h}")
                    nc.scalar.activation(
                        e_t[:, 0:w],
                        sc_ps[:, 0:w],
                        mybir.ActivationFunctionType.Exp,
                        scale=1.0 / HD,
                    )
                    nc.vector.tensor_mul(e_t[:, 0:128], e_t[:, 0:128], masksb)
                    e_pre[(c, h)] = e_t
                    if c == 0:
                        den0[h] = dpool.tile(
                            [128, ST], BF16, tag="den", name=f"den0_{h}"
                        )
                        nc.vector.tensor_copy(den0[h], e_t)
                    else:
                        nc.vector.tensor_add(
                            den0[h][:, off:ST], den0[h][:, off:ST], e_t[:, 0:w]
                        )

                pre_queue = [(c, h) for c in range(4) for h in range(hq)]

                def rope_head(acc, h, ssl, swap_dve=False):
                    # RoPE for one head; write bf16. The half-swap copies run
                    # on the scalar engine (partition-shifted copies are legal
                    # there) to cut the DVE chain to 3 ops per head — except
                    # the first head of each s-tile, whose swap goes to DVE:
                    # its acc bank gates the NEXT s-tile's first matmuls and
                    # the scalar engine is jammed with transpose copies then.
                    dst = qt_sb[h] if h < hq else kt_sb
                    t1 = rp.tile([128, ST], BF16, tag="t1")
                    nc.vector.tensor_mul(t1, acc[h], cossb[:, ssl])
                    tsw = rp.tile([128, ST], BF16, tag="tsw")
                    eng = nc.vector if swap_dve else nc.scalar
                    if swap_dve:
                        nc.vector.tensor_copy(tsw[0:64, :], acc[h][64:128, :])
                        nc.vector.tensor_copy(tsw[64:128, :], acc[h][0:64, :])
                    else:
                        nc.scalar.copy(tsw[0:64, :], acc[h][64:128, :])
                        nc.scalar.copy(tsw[64:128, :], acc[h][0:64, :])
                    nc.vector.tensor_mul(tsw, tsw, sinsb[:, ssl])
                    nc.vector.tensor_add(dst[:, ssl], t1, tsw)

                for st in range(nst):
                    ssl = slice(st * ST, (st + 1) * ST)
                    acc = [
                        psA.tile([128, ST], F32, tag="acc", name=f"acc{h}")
                        for h in range(nh)
                    ]
                    if st >= 1:
                        # s-tile seam: the first QK matmuls wait on the
                        # previous tile's RoPE to free their PSUM bank
                        # (~1us) — keep the clock gate warm through it
                        dps = psE.tile([128, ST], F32, tag="sce", name=f"dwm{st}")
                        dummy_mms(dps, 4)

                    def qk_head_quar(h, quar, x8a):
                        for i in range(npq):
                            nc.tensor.matmul(
                                acc[h],
                                w8[quar * npq + i][:, :, h * 128 : (h + 1) * 128],
                                x8a[:, i, :, :],
                                start=(quar == 0 and i == 0),
                                stop=(quar == nq - 1 and i == npq - 1),
                                perf_mode=mybir.MatmulPerfMode.DoubleRow,
                            )

                    def v_mms(quar, xta):
                        for dk in range(ndkq):
                            nc.tensor.matmul(
                                acc[nh - 1],
                                wvsb[quar * ndkq + dk],
                                xta[:, dk, :],
                                start=(quar == 0 and dk == 0),
                                stop=(quar == nq - 1 and dk == ndkq - 1),
                            )

                    def transpose_v(st):
                        # V: transpose [HD, s-tile] -> [s-chunk, HD] blocks.
                        # All copies on the scalar engine so the transpose
                        # chain (and everything behind it on the in-order PE
                        # queue) doesn't stall on the DVE RoPE backlog.
                        for j in range(ST // 128):
                            vtmp = vp.tile([128, 128], BF16, tag="vtmp")
                            nc.scalar.copy(
                                vtmp, acc[hq + 1][:, j * 128 : (j + 1) * 128]
                            )
                            tp_ps = psT.tile([128, 128], BF16, tag="tp")
                            nc.tensor.transpose(tp_ps, vtmp, identsb)
                            sc = st * (ST // 128) + j
                            nc.scalar.copy(
                                v_sb[:, sc * 128 : (sc + 1) * 128], tp_ps
                            )

                    if st < nst - 1:
                        for quar in range(nq):
                            k = st * nq + quar
                            pump(k + 4, k + 1)
                            # QK for this quarter, with the q-tile 0 attn
                            # prefetch pairs spaced between head-groups of
                            # s-tile 2; V runs one quarter STAGGERED so its
                            # xt tile and wv weights get an extra quarter of
                            # DMA slack (matters most in s-tile 0)
                            for h in range(nqk):
                                qk_head_quar(h, quar, x8_tiles[k])
                                if st == 2 and pre_queue:
                                    prefetch_attn0_pair(*pre_queue.pop(0))
                            v_mms(quar, xt_tiles[k])
                        # wo weights behind this s-tile's x loads
                        nc.sync.dma_start(
                            wosb[st], woT_d[st * 128 : (st + 1) * 128, :]
                        )
                        transpose_v(st)
                        for h in range(hq + 1):
                            rope_head(acc, h, ssl)
                    else:
                        # ---- last s-tile: head-major so each head's RoPE
                        # (and the PSUM bank phase B's attnV q-tile 0 reuses)
                        # completes while later heads still stream ----
                        pump(nquar - 1, nquar - 1)
                        nc.sync.dma_start(
                            wosb[st], woT_d[st * 128 : (st + 1) * 128, :]
                        )
                        base = st * nq
                        for h in range(hq):
                            for quar in range(nq):
                                qk_head_quar(h, quar, x8_tiles[base + quar])
                            rope_head(acc, h, ssl)
                        # q-tile 0 denominator broadcast + reciprocal, woven
                        # between the K quarter-groups so the PE never waits
                        # on the 1-deep psE ring (each bc's reciprocal runs
                        # on ACT while the next K quarter streams)
                        for quar in range(nq):
                            qk_head_quar(hq, quar, x8_tiles[base + quar])
                            bc_ps = psE.tile(
                                [128, ST], F32, tag="sce", name=f"bc0_{quar}"
                            )
                            nc.tensor.matmul(
                                bc_ps, onesrsb, den0[quar], start=True, stop=True
                            )
                            recip = sp.tile(
                                [128, ST], F32, tag="recip", name=f"recip0_{quar}"
                            )
                            lntmp = sp.tile(
                                [128, ST], F32, tag="lntmp", bufs=1,
                                name=f"lnt0_{quar}",
                            )
                            _act_reciprocal(nc, recip, bc_ps, lntmp)
                            recips0[quar] = recip
                        rope_head(acc, hq, ssl)
                        for quar in range(nq):
                            v_mms(quar, xt_tiles[base + quar])
                        transpose_v(st)

            # ============ phase B: attention + output projection ============
            # pool order matters: psAt's banks alias phase A's Q-head acc
            # banks (freed as each head's RoPE completes in the head-major
            # last s-tile), so attnV for the prefetched q-tile 0 can start
            # before the K/V epilogue finishes
            with (
                tc.tile_pool(name="psAt", bufs=4, space="PSUM") as psAt,
                tc.tile_pool(name="psS", bufs=2, space="PSUM") as psS,
                tc.tile_pool(name="psW", bufs=2, space="PSUM") as psW,
            ):
                cast_ctr = [0]
                osb_cur = [None]

                def emit_wo_task(qt, attn_tiles, j, nt):
                    # one output tile of wo for q-tile qt: 4 matmuls
                    # (contraction over the 4 heads) + cast. Four consecutive
                    # nt tiles share one [128, 2048] osb buffer flushed by a
                    # single DMA; the very last s-chunk flushes every 2 tiles
                    # to shorten the drain tail.
                    sc = qt * (ST // 128) + j
                    last_sc = sc == nst * (ST // 128) - 1
                    gran = 2 if last_sc else 4
                    o_ps = psW.tile(
                        [128, ST], F32, tag="wops", name=f"wo{qt}_{j}_{nt}"
                    )
                    for mh in range(hq):
                        nc.tensor.matmul(
                            o_ps,
                            attn_tiles[mh][:, j * 128 : (j + 1) * 128],
                            wosb[mh][:, nt * ST : (nt + 1) * ST],
                            start=(mh == 0),
                            stop=(mh == hq - 1),
                        )
                    if nt % gran == 0:
                        osb_cur[0] = op.tile(
                            [128, 4 * ST], BF16, tag="osb",
                            name=f"osb{qt}_{j}_{nt}",
                        )
                    osb = osb_cur[0]
                    # alternate the PSUM->SBUF cast between ACT and DVE
                    cast_ctr[0] += 1
                    if cast_ctr[0] % 2 == 0:
                        nc.scalar.copy(osb[:, (nt % gran) * ST : (nt % gran + 1) * ST], o_ps)
                    else:
                        nc.vector.tensor_copy(
                            osb[:, (nt % gran) * ST : (nt % gran + 1) * ST], o_ps
                        )
                    if nt % gran == gran - 1:
                        nc.sync.dma_start(
                            outp_d[
                                sc * 128 : (sc + 1) * 128,
                                (nt - gran + 1) * ST : (nt + 1) * ST,
                            ],
                            osb[:, 0 : gran * ST],
                        )

                pending = []  # wo tasks of the previous q-tile
                boundary = []  # held-back tasks two q-tiles old: already
                # normalized, so they can fill the PE during the next
                # q-tile's first chunks while everything else waits on ACT
                deferred_fin = []  # prev q-tile's recip+normalize closures
                for qt in range(nst):
                    nk = (qt + 1) * (ST // 128)  # causal: k chunks this q-tile
                    with nc.named_scope(f"attn{qt}"):
                        at_tiles = {
                            h: psAt.tile([128, ST], F32, tag="at", name=f"at{qt}_{h}")
                            for h in range(hq)
                        }
                        # den accumulation state, all off the PE:
                        # binary-counter tree over full chunks (2x-rate DVE
                        # bf16 adds; GpSimd takes one head), merged into a
                        # single running total when the full chunks end, then
                        # diagonal chunks added in place at their offset as
                        # they arrive — so the q-tile boundary only pays one
                        # add + one broadcast matmul + one reciprocal.
                        tree = {h: {} for h in range(hq)}
                        tot = {}
                        attn_tiles = {}
                        fins = []
                        if qt == 0:
                            # q-tile 0 was fully prefetched in phase A; emit
                            # its attnV head-major so head h's chain issues
                            # as soon as RoPE(Qh) frees its aliased PSUM
                            # bank, instead of gating on all four heads;
                            # dummy matmuls fragment the phase-seam stalls
                            # so the clock gate stays warm
                            for h in range(hq):
                                dummy_mms(at_tiles[h], 1)
                                if h < hq - 1:
                                    dummy_mms(at_tiles[hq - 1], 1)
                                for c in range(nk):
                                    o2 = 128 * c
                                    nc.tensor.matmul(
                                        at_tiles[h][:, o2:ST],
                                        v_sb[:, c * 128 : (c + 1) * 128],
                                        e_pre[(c, h)][:, 0 : ST - o2],
                                        start=(c == 0),
                                        stop=(c == nk - 1),
                                    )
                        for c in range(0 if qt == 0 else nk):
                            # diagonal chunks: only columns >= 128*r valid
                            r = c - (nk - 4)
                            off = 128 * r if r > 0 else 0
                            w = ST - off
                            last = c == nk - 1
                            # wo matmuls of the previous q-tile fill the PE
                            # while this chunk's exps run on the scalar
                            # engine; emit them BETWEEN the scores matmuls
                            # (the psS ring is 2 deep, so scores h=2 waits on
                            # exp h=0 — fillers keep the in-order PE queue
                            # fed). Gated until the previous q-tile's
                            # deferred normalize closures have all run: the
                            # wo matmuls read the in-place-normalized tiles.
                            fillers = []
                            if boundary and c < 2:
                                take = min(4, len(boundary))
                                fillers += [boundary.pop(0) for _ in range(take)]
                            if pending and not deferred_fin:
                                hold = 8 if qt < nst - 1 else 0
                                n_emit = -(-max(0, len(pending) - hold) // (nk - c))
                                fillers += [pending.pop(0) for _ in range(n_emit)]
                            nf2 = len(fillers) // 2
                            first_emitted = False
                            e_ts = {}
                            for h in range(hq):
                                sc_ps = psS.tile(
                                    [128, ST], F32, tag="sc",
                                    name=f"sc{qt}_{c}_{h}",
                                )
                                nc.tensor.matmul(
                                    sc_ps[:, 0:w],
                                    kt_sb[:, c * 128 : (c + 1) * 128],
                                    qt_sb[h][:, qt * ST + off : (qt + 1) * ST],
                                    start=True,
                                    stop=True,
                                )
                                e_t = ep.tile(
                                    [128, ST], BF16, tag="E",
                                    name=f"e{qt}_{c}_{h}",
                                )
                                nc.scalar.activation(
                                    e_t[:, 0:w],
                                    sc_ps[:, 0:w],
                                    mybir.ActivationFunctionType.Exp,
                                    scale=1.0 / HD,
                                )
                                if r >= 0:
                                    nc.vector.tensor_mul(
                                        e_t[:, 0:128], e_t[:, 0:128], masksb
                                    )
                                e_ts[h] = e_t
                                if h == 1:
                                    first_emitted = True
                                    for t in fillers[:nf2]:
                                        emit_wo_task(*t)
                            for t in (fillers[nf2:] if first_emitted else fillers):
                                emit_wo_task(*t)
                            if c == 0:
                                # q-tile boundary: attnV c0 trickles behind
                                # the ACT exp chain — keep the clock warm
                                for h in range(hq):
                                    dummy_mms(at_tiles[hq - 1 - h], 1)
                            for h in range(hq):
                                nc.tensor.matmul(
                                    at_tiles[h][:, off:ST],
                                    v_sb[:, c * 128 : (c + 1) * 128],
                                    e_ts[h][:, 0:w],
                                    start=(c == 0),
                                    stop=(c == nk - 1),
                                )
                                if c == 0 and h < hq - 1:
                                    dummy_mms(at_tiles[hq - 1], 1)
                            # previous q-tile's deferred reciprocal+normalize
                            # closures: two after each of the first chunks
                            if deferred_fin and c < 2:
                                for _ in range(min(2, len(deferred_fin))):
                                    deferred_fin.pop(0)()
                            # ---- denominator accumulation (off the PE) ----
                            for h in range(hq):
                                eng = nc.vector if h < 3 else nc.gpsimd
                                if off == 0:
                                    # full chunk: binary-counter tree insert
                                    carry = e_ts[h]
                                    lvl = 0
                                    while lvl < 3 and tree[h].get(lvl) is not None:
                                        t_new = tpool.tile(
                                            [128, ST], BF16, tag=f"tr{h}",
                                            name=f"tr{qt}_{c}_{h}_{lvl}",
                                        )
                                        eng.tensor_add(t_new, tree[h][lvl], carry)
                                        tree[h][lvl] = None
                                        carry = t_new
                                        lvl += 1
                                    tree[h][lvl] = carry
                                    if c == nk - 4:
                                        # full chunks end here: merge the
                                        # tree into one running total
                                        lvls = [
                                            l for l in sorted(tree[h])
                                            if tree[h][l] is not None
                                        ]
                                        t_tot = tree[h][lvls[0]]
                                        for l in lvls[1:]:
                                            t_new = tpool.tile(
                                                [128, ST], BF16, tag=f"tr{h}",
                                                name=f"tm{qt}_{c}_{h}_{l}",
                                            )
                                            eng.tensor_add(
                                                t_new, t_tot, tree[h][l]
                                            )
                                            t_tot = t_new
                                        tot[h] = t_tot
                                        tree[h] = {}
                                else:
                                    # diagonal chunk: add in place at offset
                                    eng.tensor_add(
                                        tot[h][:, off:ST],
                                        tot[h][:, off:ST],
                                        e_ts[h][:, 0:w],
                                    )
                                if last:
                                    # free the attnV PSUM bank NOW with a
                                    # plain bf16 copy (no reciprocal dep),
                                    # then defer reduce+reciprocal+in-place
                                    # normalize into the NEXT q-tile's first
                                    # chunks, where ACT interleaves them
                                    # with that q-tile's exps instead of
                                    # serializing at the boundary
                                    atn = atp.tile(
                                        [128, ST], BF16, tag="attnT",
                                        name=f"atn{qt}_{h}",
                                    )
                                    nc.vector.tensor_copy(atn, at_tiles[h])
                                    attn_tiles[h] = atn

                                    def make_fin(tot_h, atn_h, qi, hh):
                                        def fin():
                                            bc_ps = psW.tile(
                                                [128, ST], F32, tag="wops",
                                                name=f"bc{qi}_{hh}",
                                            )
                                            nc.tensor.matmul(
                                                bc_ps, onesrsb, tot_h,
                                                start=True, stop=True,
                                            )
                                            recip = sp.tile(
                                                [128, ST], F32, tag="recip",
                                                name=f"recip{qi}_{hh}",
                                            )
                                            lntmp = sp.tile(
                                                [128, ST], F32, tag="lntmp",
                                                bufs=1, name=f"lnt{qi}_{hh}",
                                            )
                                            _act_reciprocal(
                                                nc, recip, bc_ps, lntmp
                                            )
                                            nc.vector.tensor_mul(
                                                atn_h, atn_h, recip
                                            )
                                        return fin

                                    fins.append(make_fin(tot[h], atn, qt, h))
                        boundary += pending  # <=8 kept-back tasks
                        pending = []
                        if qt == 0:
                            # q-tile 0's reciprocals were computed in phase
                            # A: copy + normalize in place right away
                            for h in range(hq):
                                atn = atp.tile(
                                    [128, ST], BF16, tag="attnT",
                                    name=f"atn0_{h}",
                                )
                                nc.vector.tensor_copy(atn, at_tiles[h])
                                nc.vector.tensor_mul(atn, atn, recips0[h])
                                attn_tiles[h] = atn
                        if qt == nst - 1:
                            # no next q-tile to defer into
                            for fin in fins:
                                fin()
                            fins = []
                        deferred_fin = fins
                    pending = [
                        (qt, attn_tiles, j, nt)
                        for j in range(ST // 128)
                        for nt in range(nnt)
                    ]
                for t in boundary + pending:
                    emit_wo_task(*t)
    return _legalize_single_wait(nc)


def host_prep(x, wq, wk, wv, wo, s=S, d=D, hq=HQ, ncores=NCORES):
    """Shared tensors + per-core weight shards, all host-side numpy.

    Every big tensor is pre-swizzled into exactly the SBUF tile layout the
    kernel loads, so each dma_start moves a per-partition-contiguous block:
      xT8  [16*128, 4096]: row k*128+p, col i*1024+ko*512+n
                           <- x[d=quar*1024+i*256+ko*128+p, s=st*512+n]
      xT   [16*128, 4096]: col dk*512+n <- x[d=quar*1024+dk*128+p, ...]
      wqk8 [4*128, 5120]:  row g*128+p, col j*1280+ko*640+m
                           <- w[d=g*1024+j*256+ko*128+p, m]
      wvT  [4*128, 1024]:  col c*128+n <- w[d=g*1024+c*128+p, n]
    """
    scale = attn_scale(s, HD, MULT)
    xTf = np.ascontiguousarray(x.reshape(s, d).T)
    xT = xTf.astype(NPBF16)
    xT8 = xTf.astype(NPFP8)

    # (quar, i, ko, p, st, n) -> (st, quar, p, i, ko, n)
    x8L = np.ascontiguousarray(
        xT8.reshape(4, 4, 2, 128, 4, 512).transpose(4, 0, 3, 1, 2, 5)
    ).reshape(16 * 128, 4096)
    # (quar, dk, p, st, n) -> (st, quar, p, dk, n)
    xTL = np.ascontiguousarray(
        xT.reshape(4, 8, 128, 4, 512).transpose(3, 0, 2, 1, 4)
    ).reshape(16 * 128, 4096)

    freq = ROPE_BASE ** (-(np.arange(0, HD, 2, dtype=np.float64) / HD))
    pos = np.arange(s, dtype=np.float64)
    angle = pos[:, None] * freq[None, :]  # [s, 64]
    cos = np.cos(angle).astype(NPBF16).T  # [64, s]
    sin = np.sin(angle).astype(NPBF16).T
    cosF = np.ascontiguousarray(np.concatenate([cos, cos], axis=0))
    sinSg = np.ascontiguousarray(np.concatenate([-sin, sin], axis=0))

    # triangular causal mask for diagonal chunks: keep iff p <= f
    p = np.arange(128)[:, None]
    f = np.arange(128)[None, :]
    maskT = (p <= f).astype(NPBF16)  # [128, 128]

    ident = np.eye(128, dtype=NPBF16)
    onesr = np.ones((128, 128), dtype=NPBF16)

    shared = dict(
        xT=xTL, xT8=x8L, cosF=cosF, sinSg=sinSg, maskT=maskT, ident=ident,
        onesr=onesr,
    )

    in_maps = []
    for c in range(ncores):
        wq_c = wq[c * hq * 128 : (c + 1) * hq * 128, :]  # [hq*128, d]
        wk_c = wk[c * 128 : (c + 1) * 128, :]
        wv_c = wv[c * 128 : (c + 1) * 128, :] * scale
        wqk8 = np.ascontiguousarray(
            np.concatenate([wq_c.T, wk_c.T], axis=1)
        ).astype(NPFP8)  # [d, (hq+1)*128]
        # (g, j, ko, p, m) -> (g, p, j, ko, m)
        w8L = np.ascontiguousarray(
            wqk8.reshape(4, 4, 2, 128, 640).transpose(0, 3, 1, 2, 4)
        ).reshape(4 * 128, 5120)
        wvT = np.ascontiguousarray(wv_c.T).astype(NPBF16)  # [d, 128]
        # (g, c, p, n) -> (g, p, c, n)
        wvL = np.ascontiguousarray(
            wvT.reshape(4, 8, 128, 128).transpose(0, 2, 1, 3)
        ).reshape(4 * 128, 1024)
        wo_c = wo[:, c * hq * 128 : (c + 1) * hq * 128]  # [d, hq*128]
        woT = np.ascontiguousarray(wo_c.T).astype(NPBF16)  # [hq*128, d]
        in_maps.append(dict(shared, wqk8=w8L, wvT=wvL, woT=woT))
    return in_maps


_NC_CACHE = {}


def kernel(x, freqs_cis, wq, wk, wv, wo):
    del freqs_cis  # forward pass recomputes rope tables (matches reference)
    x = np.asarray(x, dtype=np.float32)
    key = (S, D, HQ)
    if key not in _NC_CACHE:
        _NC_CACHE[key] = build_core_kernel(S, D, HQ)
    nc = _NC_CACHE[key]
    in_maps = host_prep(
        x, np.asarray(wq, np.float32), np.asarray(wk, np.float32),
        np.asarray(wv, np.float32), np.asarray(wo, np.float32),
    )
    res = run_bass_kernel_spmd(nc, in_maps, core_ids=list(range(NCORES)))
    out = np.zeros((S, D), dtype=np.float32)
    for r in res.results:
        out += np.asarray(r["outp"], dtype=np.float32)
    return out.reshape(B, S, D)


if __name__ == "__main__":
    rng = np.random.default_rng(0)
    x = rng.standard_normal((B, S, D)).astype(np.float32)
    wq = (rng.standard_normal((H * HD, D)) * D**-0.5).astype(np.float32)
    wk = (rng.standard_normal((KVH * HD, D)) * D**-0.5).astype(np.float32)
    wv = (rng.standard_normal((KVH * HD, D)) * D**-0.5).astype(np.float32)
    wo = (rng.standard_normal((D, H * HD)) * (H * HD) ** -0.5).astype(np.float32)
    fc = rng.standard_normal((S, HD // 2)).astype(np.float32)
    out = kernel(x, fc, wq, wk, wv, wo)
    print(out.shape, out.dtype, np.abs(out).max())
